# revision 14
# baseline (speedup 1.0000x reference)
"""FNS spectral network kernel for 8x TRN2 NeuronCores (data parallel over batch).

Math (verified vs reference to ~3e-7 in fp64):
  per sample b:
    rh = (-Gi) @ r @ Gi.T          Gi[j,n] = sin(pi*(j-128)*(n+1)/256)/256, [257,255]
    x  = conv1(rh) -> conv2 -> conv3 -> *theta -> conv4 -> conv5 -> conv6
         (3x3 per-sample complex convs, pad=1; conv4..6 use _wT weights)
    e  = H @ x @ H.T               H[k,j] = exp(-2i*pi*k*(j-127)/513), [255,257]

Device mapping (1 sample/core, all weights/transforms host-preprocessed):
  - convs as block-Toeplitz matmuls: image in 43 row-blocks of 8 rows
    (stride 6, 1-row halo each side). Std layout: mid rows il=1..6 at
    partitions (il-1)*16+cp in [0:96] (cp=reim*8+ch), il0 halo at [96:112],
    il7 at [112:128]. Free = 259 cols (zero pad col each side). Per block:
    3 matmuls (dj taps via free-dim shifts), stationary T [128, 3*M].
  - T has 32 duplicated output columns ([96:112]=inn5 dup, [112:128]=inn0
    dup) so halo propagation is done by lane-aligned compute copies from
    PSUM (plain copy for il0, copy_predicated for il7) - no partition-
    shifting DMAs in the conv pipeline.
  - theta: conv3 emits (reim,inn,ch) order; DVE complex-multiplies write
    straight into conv4's input tiles (x4 uses a (reim,il,ch) layout);
    only zi lane-move + 4 small halo DMAs per block remain.
  - block 42 uses T variants with rows for image rows 257/258 zeroed, so
    out-of-image garbage partitions are never observed.
  - front/back transforms: dense matmuls using the (A@B)^T = matmul(lhsT=A,
    rhs=B) identity so no on-device transposes are needed.
  - fp16 operands (validated ~1e-3 rel err), fp32 PSUM accumulate.
  - Bacc (not raw Bass) so multi-wait instructions are legalized for TRN2.
"""

import os

import numpy as np

import concourse.bacc as bacc
import concourse.mybir as mybir
from concourse.bass_utils import run_bass_kernel_spmd
from concourse.tile import TileContext

F16 = mybir.dt.float16
F32 = mybir.dt.float32
U8 = mybir.dt.uint8

B = 8
N1 = 255
CROP = 257
CH = 8
NBLK = 43          # ceil(257/6)
WPAD = 259         # 257 cols + 1 zero col each side

LAST_EXEC_TIME_NS = None


# ----------------------------------------------------------------------------
# Host-side constant / weight preprocessing
# ----------------------------------------------------------------------------

def _host_consts():
    j = np.arange(CROP)[:, None]
    n = np.arange(N1)[None, :]
    Gi = (np.sin(np.pi * (j - 128) * (n + 1) / 256.0) / 256.0).astype(np.float32)
    k = np.arange(N1)[:, None]
    jj = np.arange(CROP)[None, :]
    H = np.exp(-2j * np.pi * k * (jj - 127.0) / 513.0)
    return {
        "g1t": np.ascontiguousarray((-Gi).T.astype(np.float16)),   # [255,257]
        "g2t": np.ascontiguousarray(Gi.T.astype(np.float16)),      # [255,257]
        "hrt": np.ascontiguousarray(H.real.T.astype(np.float16)),  # [257,255]
        "hit": np.ascontiguousarray(H.imag.T.astype(np.float16)),  # [257,255]
        "hnit": np.ascontiguousarray((-H.imag).T.astype(np.float16)),
    }


def _expand_w(wre, wim):
    """[Co,Ci,3,3] complex -> real packed [2Co, 2Ci, 3, 3], part = reim*C+ch."""
    Co, Ci = wre.shape[0], wre.shape[1]
    W = np.zeros((2 * Co, 2 * Ci, 3, 3), np.float32)
    W[:Co, :Ci] = wre
    W[:Co, Ci:] = -wim
    W[Co:, :Ci] = wim
    W[Co:, Ci:] = wre
    return W


def _wT(wre, wim):
    """torch _wT: swap cout/cin, transpose 3x3 kernel, conjugate."""
    wre2 = np.swapaxes(np.swapaxes(wre, 0, 1), -2, -1)
    wim2 = -np.swapaxes(np.swapaxes(wim, 0, 1), -2, -1)
    return wre2, wim2


def _row_std(p):
    if p < 96:
        return 1 + p // 16, p % 16
    if p < 112:
        return 0, p - 96
    return 7, p - 112


def _row_x4(p):
    """x4 (theta output) layout: re mid [0:48], il0 halos [48:64] (re,im),
    im mid [64:112], il7 halos [112:128] (re,im)."""
    if p < 48:
        return 1 + p // 8, p % 8
    if p < 64:
        return 0, p - 48         # [48:56] re ch, [56:64] im ch
    if p < 112:
        q = p - 64
        return 1 + q // 8, 8 + q % 8
    return 7, p - 112            # [112:120] re ch, [120:128] im ch


def _col_std_dup(m):
    """M=128 col map with dup halo outputs: [0:96] mid, [96:112] inn5 dup,
    [112:128] inn0 dup."""
    if m < 96:
        return m // 16, m % 16
    if m < 112:
        return 5, m - 96
    return 0, m - 112


def _build_T(Wexp, rowmap, colmap, K, M, zero42=False):
    T = np.zeros((K, 3 * M), np.float32)
    Cin2 = Wexp.shape[1]
    for p in range(K):
        il, cp = rowmap(p)
        if cp >= Cin2:
            continue
        if zero42 and il >= 6:
            continue
        for dj in range(3):
            for m in range(M):
                inn, op = colmap(m)
                di = il - inn
                if 0 <= di <= 2:
                    T[p, dj * M + m] = Wexp[op, cp, di, dj]
    return T.astype(np.float16)


def _host_prep_sample(bidx, inputs, consts):
    s = {}
    s["r16"] = np.ascontiguousarray(inputs["r"][bidx, 0].astype(np.float16))
    s.update(consts)

    w1 = (inputs["w1_re"][bidx], inputs["w1_im"][bidx])  # [8,1,3,3]
    w2 = (inputs["w2_re"][bidx], inputs["w2_im"][bidx])
    w3 = (inputs["w3_re"][bidx], inputs["w3_im"][bidx])

    W1r = _expand_w(*w1)[:, 0:1]          # [16, 1, 3, 3] (input is real rh)
    W2 = _expand_w(*w2)
    W3 = _expand_w(*w3)
    W4 = _expand_w(*_wT(*w3))
    W5 = _expand_w(*_wT(*w2))
    W6 = _expand_w(*_wT(*w1))             # [2, 16, 3, 3]

    def col_c3(m):
        reim, inn, ch = m // 48, (m % 48) // 8, m % 8
        return inn, reim * 8 + ch

    def col_c6(m):
        return m % 6, m // 6

    def row_x1(p):
        return p, 0

    s["t1"] = _build_T(W1r, row_x1, _col_std_dup, 8, 128)
    s["t2"] = _build_T(W2, _row_std, _col_std_dup, 128, 128)
    s["t2b"] = _build_T(W2, _row_std, _col_std_dup, 128, 128, zero42=True)
    s["t3"] = _build_T(W3, _row_std, col_c3, 128, 96)
    s["t3b"] = _build_T(W3, _row_std, col_c3, 128, 96, zero42=True)
    s["t4"] = _build_T(W4, _row_x4, _col_std_dup, 128, 128)
    s["t4b"] = _build_T(W4, _row_x4, _col_std_dup, 128, 128, zero42=True)
    s["t5"] = _build_T(W5, _row_std, _col_std_dup, 128, 128)
    s["t5b"] = _build_T(W5, _row_std, _col_std_dup, 128, 128, zero42=True)
    s["t6"] = _build_T(W6, _row_std, col_c6, 128, 12)
    s["t6b"] = _build_T(W6, _row_std, col_c6, 128, 12, zero42=True)

    # theta per-block tiles: [48, NBLK * 2*257], block b re at +0, im at +257;
    # row = inn*8+ch matching conv3's (reim,inn,ch) output order
    th = np.zeros((48, NBLK * 2 * CROP), np.float16)
    tr = inputs["theta_re"][bidx]  # [8, 257, 257]
    ti = inputs["theta_im"][bidx]
    for b in range(NBLK):
        ninn = 6 if b < NBLK - 1 else 5
        base = b * 2 * CROP
        for inn in range(ninn):
            row = 6 * b + inn
            for ch in range(CH):
                th[inn * 8 + ch, base:base + CROP] = tr[ch, row]
                th[inn * 8 + ch, base + CROP:base + 2 * CROP] = ti[ch, row]
    s["thet"] = th
    return s


# ----------------------------------------------------------------------------
# Device program
# ----------------------------------------------------------------------------

def _build_nc():
    nc = bacc.Bacc(None, target_bir_lowering=False, debug=False)

    dp = {}
    for name, shape, dt in (
        ("r16", [N1, N1], F16), ("g1t", [N1, CROP], F16),
        ("g2t", [N1, CROP], F16), ("hrt", [CROP, N1], F16),
        ("hit", [CROP, N1], F16), ("hnit", [CROP, N1], F16),
        ("t1", [8, 384], F16), ("t2", [128, 384], F16),
        ("t2b", [128, 384], F16), ("t3", [128, 288], F16),
        ("t3b", [128, 288], F16), ("t4", [128, 384], F16),
        ("t4b", [128, 384], F16), ("t5", [128, 384], F16),
        ("t5b", [128, 384], F16), ("t6", [128, 36], F16),
        ("t6b", [128, 36], F16), ("thet", [48, NBLK * 2 * CROP], F16),
    ):
        dp[name] = nc.declare_dram_parameter(name, list(shape), dt,
                                             isOutput=False)
    ere = nc.declare_dram_parameter("ere", [N1, N1], F32, isOutput=True)
    eim = nc.declare_dram_parameter("eim", [N1, N1], F32, isOutput=True)

    with TileContext(nc) as tc:
        with (
            tc.tile_pool(name="const", bufs=1) as pc,
            tc.tile_pool(name="xbuf", bufs=1) as px,
            tc.tile_pool(name="work", bufs=1) as pw,
            tc.tile_pool(name="wk2", bufs=3) as pw2,
            tc.tile_pool(name="psum", bufs=8, space="PSUM") as pp,
        ):
            # ---------------- constant loads ----------------
            def load_const(name):
                shape = [int(x) for x in dp[name].shape]
                t = pc.tile(shape, F16, name=name, tag=name)
                nc.sync.dma_start(t[:, :], dp[name][:, :])
                return t

            def load_chunks(name, rows, cols):
                out = []
                r0 = 0
                while r0 < rows:
                    rr = min(128, rows - r0)
                    t = pc.tile([rr, cols], F16, name=f"{name}{r0}",
                                tag=f"{name}{r0}")
                    nc.sync.dma_start(t[:, :], dp[name][r0:r0 + rr, :])
                    out.append(t)
                    r0 += rr
                return out

            r_sb = load_chunks("r16", N1, N1)            # [128,255],[127,255]
            g1_sb = load_chunks("g1t", N1, CROP)
            g2_sb = load_chunks("g2t", N1, CROP)
            hr_sb = load_chunks("hrt", CROP, N1)         # [128],[128],[1]
            hi_sb = load_chunks("hit", CROP, N1)
            hn_sb = load_chunks("hnit", CROP, N1)
            tsb = {k: load_const(k) for k in
                   ("t1", "t2", "t2b", "t3", "t3b", "t4", "t4b",
                    "t5", "t5b", "t6", "t6b")}

            # ---------------- X block buffers (write-once) ----------------
            x1 = [px.tile([8, WPAD], F16, name=f"x1_{b}", tag=f"x1_{b}")
                  for b in range(NBLK)]
            xs = {}
            for li in ("2", "3", "4", "5", "6"):
                xs[li] = [px.tile([128, WPAD], F16, name=f"x{li}_{b}",
                                  tag=f"x{li}_{b}") for b in range(NBLK)]

            # predication mask for il7-halo copies: 1 on [112:128], 0 on [96:112]
            mk = pw.tile([128, CROP], U8, name="mk", tag="mk")
            nc.gpsimd.memset(mk[96:128, :], 1.0)
            nc.gpsimd.memset(mk[96:112, :], 0.0)

            for b in range(NBLK):
                nc.vector.memset(x1[b][:, :], 0.0)
            for li, tiles in xs.items():
                for b in range(NBLK):
                    nc.gpsimd.memset(tiles[b][:, 0:1], 0.0)
                    nc.gpsimd.memset(tiles[b][:, 258:259], 0.0)
                if li == "4":
                    # il0 zones live at [48:64]; start-48 is illegal so clear
                    # [32:64] (the [32:48] part is later overwritten by theta)
                    nc.gpsimd.memset(tiles[0][32:64, :], 0.0)
                else:
                    nc.gpsimd.memset(tiles[0][96:112, :], 0.0)  # il0 of blk 0
                # block 42: il6/il7 zones are never written (T*b zeroes their
                # weights); full memset keeps reads initialized
                nc.vector.memset(tiles[NBLK - 1][:, :], 0.0)

            # ---------------- front transform ----------------
            # Vt = r^T @ G1^T = (G1 r)^T   [255, 257]
            vt_sb = [pw.tile([128, CROP], F16, name="vt0", tag="vt0"),
                     pw.tile([127, CROP], F16, name="vt1", tag="vt1")]
            for m, (m0, mm) in enumerate(((0, 128), (128, 127))):
                ps = pp.tile([128, CROP], F32, name="ps", tag="ps")
                for k2 in range(2):
                    nc.tensor.matmul(
                        ps[0:mm, :], lhsT=r_sb[k2][:, m0:m0 + mm],
                        rhs=g1_sb[k2][:, :], start=(k2 == 0), stop=(k2 == 1))
                nc.scalar.copy(vt_sb[m][:, :], ps[0:mm, :])

            # rh = Vt^T @ G2^T = G1 r G2^T   [257, 257]
            rh_sb = [pw.tile([128, CROP], F16, name="rh0", tag="rh0"),
                     pw.tile([128, CROP], F16, name="rh1", tag="rh1"),
                     pw.tile([1, CROP], F16, name="rh2", tag="rh2")]
            for m, (m0, mm) in enumerate(((0, 128), (128, 128), (256, 1))):
                ps = pp.tile([128, CROP], F32, name="ps", tag="ps")
                for k2 in range(2):
                    nc.tensor.matmul(
                        ps[0:mm, :], lhsT=vt_sb[k2][:, m0:m0 + mm],
                        rhs=g2_sb[k2][:, :], start=(k2 == 0), stop=(k2 == 1))
                nc.vector.tensor_copy(rh_sb[m][:, :], ps[0:mm, :])

            # scatter rh rows into conv1 input blocks
            for b in range(NBLK):
                lo = max(0, 6 * b - 1)
                hi = min(256, 6 * b + 6)
                r0 = lo
                while r0 <= hi:
                    c = r0 // 128
                    c_end = min(hi, c * 128 + 127)
                    cnt = c_end - r0 + 1
                    il0 = r0 - (6 * b - 1)
                    nc.sync.dma_start(
                        x1[b][il0:il0 + cnt, 1:258],
                        rh_sb[c][r0 - c * 128:r0 - c * 128 + cnt, :])
                    r0 = c_end + 1

            # ---------------- conv layers ----------------
            def conv_matmuls(tkey, xin_b, M, b):
                key = tkey + "b" if (b == NBLK - 1 and tkey != "t1") else tkey
                t = tsb[key]
                ps = pp.tile([128, CROP], F32, name="ps", tag="ps")
                for dj in range(3):
                    nc.tensor.matmul(
                        ps[0:M, :], lhsT=t[:, dj * M:(dj + 1) * M],
                        rhs=xin_b[:, dj:dj + CROP],
                        start=(dj == 0), stop=(dj == 2))
                return ps

            def evict_zones(ps, xout, b):
                hi = 96 if b < NBLK - 1 else 80
                # alternate engines per block to double eviction throughput
                if b % 2:
                    nc.scalar.copy(xout[b][0:hi, 1:258], ps[0:hi, :])
                    zeng = nc.vector
                else:
                    nc.vector.tensor_copy(xout[b][0:hi, 1:258], ps[0:hi, :])
                    zeng = nc.scalar
                # il0 of block b+1 (= this block's inn5 dup)
                if b + 1 < NBLK:
                    zeng.copy(xout[b + 1][96:112, 1:258], ps[96:112, :]) \
                        if zeng is nc.scalar else \
                        zeng.tensor_copy(xout[b + 1][96:112, 1:258],
                                         ps[96:112, :])
                # il7 of block b-1 (= this block's inn0 dup, masked; DVE only)
                if b > 0:
                    nc.vector.copy_predicated(xout[b - 1][96:128, 1:258],
                                              mk[96:128, 0:257],
                                              ps[96:128, :])

            def conv_layer(tkey, xin, xout, kin):
                for b in range(NBLK):
                    xi = xin[b][0:kin, :] if kin < 128 else xin[b][:, :]
                    ps = conv_matmuls(tkey, xi, 128, b)
                    evict_zones(ps, xout, b)

            conv_layer("t1", x1, xs["2"], 8)         # conv1
            conv_layer("t2", xs["2"], xs["3"], 128)  # conv2

            # conv3 + theta: xs3 -> xs4 (x4 layout: re [0:48], im [48:96])
            x4 = xs["4"]
            for b in range(NBLK):
                ps = conv_matmuls("t3", xs["3"][b][:, :], 96, b)
                z = pw2.tile([96, CROP], F32, name="z", tag="z")
                nc.scalar.copy(z[:, :], ps[0:96, :])
                zmv = pw2.tile([48, CROP], F32, name="zmv", tag="zmv")
                nc.sync.dma_start(zmv[:, :], z[48:96, :])
                tht = pw2.tile([48, 2 * CROP], F16, name="tht", tag="tht",
                               bufs=8)
                nc.sync.dma_start(tht[:, :],
                                  dp["thet"][:, b * 2 * CROP:(b + 1) * 2 * CROP])
                thr = tht[0:48, 0:CROP]
                thi = tht[0:48, CROP:2 * CROP]
                m1 = pw2.tile([48, CROP], F32, name="m1", tag="m1")
                m2 = pw2.tile([48, CROP], F32, name="m2", tag="m2")
                m3 = pw2.tile([48, CROP], F32, name="m3", tag="m3")
                m4 = pw2.tile([48, CROP], F32, name="m4", tag="m4")
                nc.vector.tensor_mul(m1[:, :], z[0:48, :], thr)
                nc.vector.tensor_mul(m2[:, :], zmv[:, :], thi)
                nc.vector.tensor_sub(x4[b][0:48, 1:258], m1[:, :], m2[:, :])
                nc.vector.tensor_mul(m3[:, :], z[0:48, :], thi)
                nc.vector.tensor_mul(m4[:, :], zmv[:, :], thr)
                nc.vector.tensor_add(x4[b][64:112, 1:258], m3[:, :], m4[:, :])
                # halos via small DMAs (x4 layout is lane-incompatible)
                if b > 0:
                    nc.sync.dma_start(x4[b][48:56, 1:258],
                                      x4[b - 1][40:48, 1:258])
                    nc.sync.dma_start(x4[b][56:64, 1:258],
                                      x4[b - 1][104:112, 1:258])
                    nc.sync.dma_start(x4[b - 1][112:120, 1:258],
                                      x4[b][0:8, 1:258])
                    nc.sync.dma_start(x4[b - 1][120:128, 1:258],
                                      x4[b][64:72, 1:258])

            conv_layer("t4", xs["4"], xs["5"], 128)  # conv4
            conv_layer("t5", xs["5"], xs["6"], 128)  # conv5

            # conv6: xs6 -> Xout planes [257, 257] re/im (3 chunks each)
            xo = {}
            for p in ("re", "im"):
                xo[p] = [pw.tile([128, CROP], F16, name=f"xo{p}0", tag=f"xo{p}0"),
                         pw.tile([128, CROP], F16, name=f"xo{p}1", tag=f"xo{p}1"),
                         pw.tile([1, CROP], F16, name=f"xo{p}2", tag=f"xo{p}2")]
            for b in range(NBLK):
                ps = conv_matmuls("t6", xs["6"][b][:, :], 12, b)
                y6 = pw2.tile([12, CROP], F16, name="y6", tag="y6", bufs=4)
                nc.scalar.copy(y6[:, :], ps[0:12, :])
                nil = 6 if b < NBLK - 1 else 5
                for pi, p in enumerate(("re", "im")):
                    r0 = 6 * b
                    while r0 < 6 * b + nil:
                        c = r0 // 128
                        c_end = min(6 * b + nil - 1, c * 128 + 127)
                        cnt = c_end - r0 + 1
                        nc.sync.dma_start(
                            xo[p][c][r0 - c * 128:r0 - c * 128 + cnt, :],
                            y6[pi * 6 + (r0 - 6 * b):pi * 6 + (r0 - 6 * b) + cnt, :])
                        r0 = c_end + 1

            # ---------------- back transform ----------------
            at = {}
            for p in ("re", "im"):
                at[p] = [pw.tile([128, N1], F16, name=f"at{p}0", tag=f"at{p}0"),
                         pw.tile([128, N1], F16, name=f"at{p}1", tag=f"at{p}1"),
                         pw.tile([1, N1], F16, name=f"at{p}2", tag=f"at{p}2")]
            for m, (m0, mm) in enumerate(((0, 128), (128, 128), (256, 1))):
                for p, terms in (("re", (("re", hr_sb), ("im", hn_sb))),
                                 ("im", (("re", hi_sb), ("im", hr_sb)))):
                    ps = pp.tile([128, N1], F32, name="ps", tag="ps")
                    nmm = 0
                    for (xp, hsb) in terms:
                        for k2 in range(3):
                            nc.tensor.matmul(
                                ps[0:mm, :],
                                lhsT=xo[xp][k2][:, m0:m0 + mm],
                                rhs=hsb[k2][:, :],
                                start=(nmm == 0), stop=(nmm == 5))
                            nmm += 1
                    nc.scalar.copy(at[p][m][:, :], ps[0:mm, :])

            e_sb = {}
            for p in ("re", "im"):
                e_sb[p] = [pw.tile([128, N1], F32, name=f"e{p}0", tag=f"e{p}0"),
                           pw.tile([127, N1], F32, name=f"e{p}1", tag=f"e{p}1")]
            for m, (m0, mm) in enumerate(((0, 128), (128, 127))):
                for p, terms in (("re", (("re", hr_sb), ("im", hn_sb))),
                                 ("im", (("re", hi_sb), ("im", hr_sb)))):
                    ps = pp.tile([128, N1], F32, name="ps", tag="ps")
                    nmm = 0
                    for (ap_, hsb) in terms:
                        for k2 in range(3):
                            nc.tensor.matmul(
                                ps[0:mm, :],
                                lhsT=at[ap_][k2][:, m0:m0 + mm],
                                rhs=hsb[k2][:, :],
                                start=(nmm == 0), stop=(nmm == 5))
                            nmm += 1
                    nc.vector.tensor_copy(e_sb[p][m][:, :], ps[0:mm, :])

            for p, dram in (("re", ere), ("im", eim)):
                nc.sync.dma_start(dram[0:128, :], e_sb[p][0][:, :])
                nc.sync.dma_start(dram[128:255, :], e_sb[p][1][:, :])

    nc.finalize()
    return nc


_NC_CACHE = None


def _get_nc():
    global _NC_CACHE
    if _NC_CACHE is None:
        _NC_CACHE = _build_nc()
    return _NC_CACHE


def kernel(**inputs):
    global LAST_EXEC_TIME_NS
    inputs = {k: np.asarray(v) for k, v in inputs.items()}
    consts = _host_consts()
    in_maps = [_host_prep_sample(b, inputs, consts) for b in range(B)]
    nc = _get_nc()
    trace = bool(os.environ.get("BASS_TRACE"))
    res = run_bass_kernel_spmd(nc, in_maps, list(range(B)), trace=trace)
    LAST_EXEC_TIME_NS = res.exec_time_ns
    out = np.zeros((B, 1, N1, N1), np.complex64)
    for b in range(B):
        out[b, 0] = res.results[b]["ere"] + 1j * res.results[b]["eim"]
    return out


# revision 16
# speedup vs baseline: 1.0396x; 1.0396x over previous
"""FNS spectral network kernel for 8x TRN2 NeuronCores (data parallel over batch).

Math (verified vs reference to ~3e-7 in fp64):
  per sample b:
    rh = (-Gi) @ r @ Gi.T          Gi[j,n] = sin(pi*(j-128)*(n+1)/256)/256, [257,255]
    x  = conv1(rh) -> conv2 -> conv3 -> *theta -> conv4 -> conv5 -> conv6
         (3x3 per-sample complex convs, pad=1; conv4..6 use _wT weights)
    e  = H @ x @ H.T               H[k,j] = exp(-2i*pi*k*(j-127)/513), [255,257]

Device mapping (1 sample/core, all weights/transforms host-preprocessed):
  - convs as block-Toeplitz matmuls: image in 43 row-blocks of 8 rows
    (stride 6, 1-row halo each side). Std layout: mid rows il=1..6 at
    partitions (il-1)*16+cp in [0:96] (cp=reim*8+ch), il0 halo at [96:112],
    il7 at [112:128]. Free = 259 cols (zero pad col each side). Per block:
    3 matmuls (dj taps via free-dim shifts), stationary T [128, 3*M].
  - T has 32 duplicated output columns ([96:112]=inn5 dup, [112:128]=inn0
    dup) so halo propagation is done by lane-aligned compute copies from
    PSUM (plain copy for il0, copy_predicated for il7) - no partition-
    shifting DMAs in the conv pipeline.
  - theta: conv3 emits (reim,inn,ch) order; DVE complex-multiplies write
    straight into conv4's input tiles (x4 uses a (reim,il,ch) layout);
    only zi lane-move + 4 small halo DMAs per block remain.
  - block 42 uses T variants with rows for image rows 257/258 zeroed, so
    out-of-image garbage partitions are never observed.
  - front/back transforms: dense matmuls using the (A@B)^T = matmul(lhsT=A,
    rhs=B) identity so no on-device transposes are needed.
  - fp16 operands (validated ~1e-3 rel err), fp32 PSUM accumulate.
  - Bacc (not raw Bass) so multi-wait instructions are legalized for TRN2.
"""

import os

import numpy as np

import concourse.bacc as bacc
import concourse.mybir as mybir
from concourse.bass_utils import run_bass_kernel_spmd
from concourse.tile import TileContext

F16 = mybir.dt.float16
F32 = mybir.dt.float32
U8 = mybir.dt.uint8

B = 8
N1 = 255
CROP = 257
CH = 8
NBLK = 43          # ceil(257/6)
WPAD = 259         # 257 cols + 1 zero col each side

LAST_EXEC_TIME_NS = None


# ----------------------------------------------------------------------------
# Host-side constant / weight preprocessing
# ----------------------------------------------------------------------------

def _host_consts():
    j = np.arange(CROP)[:, None]
    n = np.arange(N1)[None, :]
    Gi = (np.sin(np.pi * (j - 128) * (n + 1) / 256.0) / 256.0).astype(np.float32)
    k = np.arange(N1)[:, None]
    jj = np.arange(CROP)[None, :]
    H = np.exp(-2j * np.pi * k * (jj - 127.0) / 513.0)
    return {
        "g1t": np.ascontiguousarray((-Gi).T.astype(np.float16)),   # [255,257]
        "g2t": np.ascontiguousarray(Gi.T.astype(np.float16)),      # [255,257]
        "hrt": np.ascontiguousarray(H.real.T.astype(np.float16)),  # [257,255]
        "hit": np.ascontiguousarray(H.imag.T.astype(np.float16)),  # [257,255]
        "hnit": np.ascontiguousarray((-H.imag).T.astype(np.float16)),
    }


def _expand_w(wre, wim):
    """[Co,Ci,3,3] complex -> real packed [2Co, 2Ci, 3, 3], part = reim*C+ch."""
    Co, Ci = wre.shape[0], wre.shape[1]
    W = np.zeros((2 * Co, 2 * Ci, 3, 3), np.float32)
    W[:Co, :Ci] = wre
    W[:Co, Ci:] = -wim
    W[Co:, :Ci] = wim
    W[Co:, Ci:] = wre
    return W


def _wT(wre, wim):
    """torch _wT: swap cout/cin, transpose 3x3 kernel, conjugate."""
    wre2 = np.swapaxes(np.swapaxes(wre, 0, 1), -2, -1)
    wim2 = -np.swapaxes(np.swapaxes(wim, 0, 1), -2, -1)
    return wre2, wim2


def _row_std(p):
    if p < 96:
        return 1 + p // 16, p % 16
    if p < 112:
        return 0, p - 96
    return 7, p - 112


def _row_x4(p):
    """x4 (theta output) layout: re mid [0:48], il0 halos [48:64] (re,im),
    im mid [64:112], il7 halos [112:128] (re,im)."""
    if p < 48:
        return 1 + p // 8, p % 8
    if p < 64:
        return 0, p - 48         # [48:56] re ch, [56:64] im ch
    if p < 112:
        q = p - 64
        return 1 + q // 8, 8 + q % 8
    return 7, p - 112            # [112:120] re ch, [120:128] im ch


def _col_std_dup(m):
    """M=128 col map with dup halo outputs: [0:96] mid, [96:112] inn5 dup,
    [112:128] inn0 dup."""
    if m < 96:
        return m // 16, m % 16
    if m < 112:
        return 5, m - 96
    return 0, m - 112


def _build_T(Wexp, rowmap, colmap, K, M, zero42=False):
    T = np.zeros((K, 3 * M), np.float32)
    Cin2 = Wexp.shape[1]
    for p in range(K):
        il, cp = rowmap(p)
        if cp >= Cin2:
            continue
        if zero42 and il >= 6:
            continue
        for dj in range(3):
            for m in range(M):
                co = colmap(m)
                if co is None:
                    continue
                inn, op = co
                di = il - inn
                if 0 <= di <= 2:
                    T[p, dj * M + m] = Wexp[op, cp, di, dj]
    return T.astype(np.float16)


def _host_prep_sample(bidx, inputs, consts):
    s = {}
    s["r16"] = np.ascontiguousarray(inputs["r"][bidx, 0].astype(np.float16))
    s.update(consts)

    w1 = (inputs["w1_re"][bidx], inputs["w1_im"][bidx])  # [8,1,3,3]
    w2 = (inputs["w2_re"][bidx], inputs["w2_im"][bidx])
    w3 = (inputs["w3_re"][bidx], inputs["w3_im"][bidx])

    W1r = _expand_w(*w1)[:, 0:1]          # [16, 1, 3, 3] (input is real rh)
    W2 = _expand_w(*w2)
    W3 = _expand_w(*w3)
    W4 = _expand_w(*_wT(*w3))
    W5 = _expand_w(*_wT(*w2))
    W6 = _expand_w(*_wT(*w1))             # [2, 16, 3, 3]

    def col_c3a(m):
        if m < 48:
            return m // 8, m % 8          # re
        if 64 <= m < 112:
            q = m - 64
            return q // 8, 8 + q % 8      # im
        return None

    def col_c3b(m):
        if m < 48:
            return m // 8, 8 + m % 8      # im
        if 64 <= m < 112:
            q = m - 64
            return q // 8, q % 8          # re
        return None

    def col_c6(m):
        return m % 6, m // 6

    def row_x1(p):
        return p, 0

    s["t1"] = _build_T(W1r, row_x1, _col_std_dup, 8, 128)
    s["t2"] = _build_T(W2, _row_std, _col_std_dup, 128, 128)
    s["t2b"] = _build_T(W2, _row_std, _col_std_dup, 128, 128, zero42=True)
    s["t3"] = _build_T(W3, _row_std, col_c3a, 128, 128)
    s["t3b"] = _build_T(W3, _row_std, col_c3a, 128, 128, zero42=True)
    s["t3s"] = _build_T(W3, _row_std, col_c3b, 128, 128)
    s["t3sb"] = _build_T(W3, _row_std, col_c3b, 128, 128, zero42=True)
    s["t4"] = _build_T(W4, _row_x4, _col_std_dup, 128, 128)
    s["t4b"] = _build_T(W4, _row_x4, _col_std_dup, 128, 128, zero42=True)
    s["t5"] = _build_T(W5, _row_std, _col_std_dup, 128, 128)
    s["t5b"] = _build_T(W5, _row_std, _col_std_dup, 128, 128, zero42=True)
    s["t6"] = _build_T(W6, _row_std, col_c6, 128, 12)
    s["t6b"] = _build_T(W6, _row_std, col_c6, 128, 12, zero42=True)

    # theta per-block tiles: [48, NBLK * 2*257], block b re at +0, im at +257;
    # row = inn*8+ch matching conv3's (reim,inn,ch) output order
    th = np.zeros((128, NBLK * 2 * CROP), np.float16)
    tr = inputs["theta_re"][bidx]  # [8, 257, 257]
    ti = inputs["theta_im"][bidx]
    for b in range(NBLK):
        ninn = 6 if b < NBLK - 1 else 5
        base = b * 2 * CROP
        for inn in range(ninn):
            row = 6 * b + inn
            for ch in range(CH):
                p = inn * 8 + ch
                th[p, base:base + CROP] = tr[ch, row]
                th[64 + p, base:base + CROP] = tr[ch, row]
                th[p, base + CROP:base + 2 * CROP] = ti[ch, row]
                th[64 + p, base + CROP:base + 2 * CROP] = ti[ch, row]
    s["thet"] = th
    return s


# ----------------------------------------------------------------------------
# Device program
# ----------------------------------------------------------------------------

def _build_nc():
    nc = bacc.Bacc(None, target_bir_lowering=False, debug=False)

    dp = {}
    for name, shape, dt in (
        ("r16", [N1, N1], F16), ("g1t", [N1, CROP], F16),
        ("g2t", [N1, CROP], F16), ("hrt", [CROP, N1], F16),
        ("hit", [CROP, N1], F16), ("hnit", [CROP, N1], F16),
        ("t1", [8, 384], F16), ("t2", [128, 384], F16),
        ("t2b", [128, 384], F16), ("t3", [128, 384], F16),
        ("t3b", [128, 384], F16),
        ("t3s", [128, 384], F16), ("t3sb", [128, 384], F16), ("t4", [128, 384], F16),
        ("t4b", [128, 384], F16), ("t5", [128, 384], F16),
        ("t5b", [128, 384], F16), ("t6", [128, 36], F16),
        ("t6b", [128, 36], F16), ("thet", [128, NBLK * 2 * CROP], F16),
    ):
        dp[name] = nc.declare_dram_parameter(name, list(shape), dt,
                                             isOutput=False)
    ere = nc.declare_dram_parameter("ere", [N1, N1], F32, isOutput=True)
    eim = nc.declare_dram_parameter("eim", [N1, N1], F32, isOutput=True)

    with TileContext(nc) as tc:
        with (
            tc.tile_pool(name="const", bufs=1) as pc,
            tc.tile_pool(name="xbuf", bufs=1) as px,
            tc.tile_pool(name="work", bufs=1) as pw,
            tc.tile_pool(name="wk2", bufs=3) as pw2,
            tc.tile_pool(name="psum", bufs=8, space="PSUM") as pp,
        ):
            # ---------------- constant loads ----------------
            def load_const(name):
                shape = [int(x) for x in dp[name].shape]
                t = pc.tile(shape, F16, name=name, tag=name)
                nc.sync.dma_start(t[:, :], dp[name][:, :])
                return t

            def load_chunks(name, rows, cols):
                out = []
                r0 = 0
                while r0 < rows:
                    rr = min(128, rows - r0)
                    t = pc.tile([rr, cols], F16, name=f"{name}{r0}",
                                tag=f"{name}{r0}")
                    nc.sync.dma_start(t[:, :], dp[name][r0:r0 + rr, :])
                    out.append(t)
                    r0 += rr
                return out

            r_sb = load_chunks("r16", N1, N1)            # [128,255],[127,255]
            g1_sb = load_chunks("g1t", N1, CROP)
            g2_sb = load_chunks("g2t", N1, CROP)
            hr_sb = load_chunks("hrt", CROP, N1)         # [128],[128],[1]
            hi_sb = load_chunks("hit", CROP, N1)
            hn_sb = load_chunks("hnit", CROP, N1)
            tsb = {k: load_const(k) for k in
                   ("t1", "t2", "t2b", "t3", "t3b", "t4", "t4b",
                    "t5", "t5b", "t6", "t6b", "t3s", "t3sb")}

            # ---------------- X block buffers (write-once) ----------------
            x1 = [px.tile([8, WPAD], F16, name=f"x1_{b}", tag=f"x1_{b}")
                  for b in range(NBLK)]
            xs = {}
            for li in ("2", "3", "4", "5", "6"):
                xs[li] = [px.tile([128, WPAD], F16, name=f"x{li}_{b}",
                                  tag=f"x{li}_{b}") for b in range(NBLK)]

            # predication mask for il7-halo copies: 1 on [112:128], 0 on [96:112]
            mk = pw.tile([128, CROP], U8, name="mk", tag="mk")
            nc.gpsimd.memset(mk[96:128, :], 1.0)
            nc.gpsimd.memset(mk[96:112, :], 0.0)

            for b in range(NBLK):
                nc.vector.memset(x1[b][:, :], 0.0)
            for li, tiles in xs.items():
                for b in range(NBLK):
                    nc.gpsimd.memset(tiles[b][:, 0:1], 0.0)
                    nc.gpsimd.memset(tiles[b][:, 258:259], 0.0)
                if li == "4":
                    # il0 zones live at [48:64]; start-48 is illegal so clear
                    # [32:64] (the [32:48] part is later overwritten by theta)
                    nc.gpsimd.memset(tiles[0][32:64, :], 0.0)
                else:
                    nc.gpsimd.memset(tiles[0][96:112, :], 0.0)  # il0 of blk 0
                # block 42: il6/il7 zones are never written (T*b zeroes their
                # weights); full memset keeps reads initialized
                nc.vector.memset(tiles[NBLK - 1][:, :], 0.0)

            # ---------------- front transform ----------------
            # Vt = r^T @ G1^T = (G1 r)^T   [255, 257]
            vt_sb = [pw.tile([128, CROP], F16, name="vt0", tag="vt0"),
                     pw.tile([127, CROP], F16, name="vt1", tag="vt1")]
            for m, (m0, mm) in enumerate(((0, 128), (128, 127))):
                ps = pp.tile([128, CROP], F32, name="ps", tag="ps")
                for k2 in range(2):
                    nc.tensor.matmul(
                        ps[0:mm, :], lhsT=r_sb[k2][:, m0:m0 + mm],
                        rhs=g1_sb[k2][:, :], start=(k2 == 0), stop=(k2 == 1))
                nc.scalar.copy(vt_sb[m][:, :], ps[0:mm, :])

            # rh = Vt^T @ G2^T = G1 r G2^T   [257, 257]
            rh_sb = [pw.tile([128, CROP], F16, name="rh0", tag="rh0"),
                     pw.tile([128, CROP], F16, name="rh1", tag="rh1"),
                     pw.tile([1, CROP], F16, name="rh2", tag="rh2")]
            for m, (m0, mm) in enumerate(((0, 128), (128, 128), (256, 1))):
                ps = pp.tile([128, CROP], F32, name="ps", tag="ps")
                for k2 in range(2):
                    nc.tensor.matmul(
                        ps[0:mm, :], lhsT=vt_sb[k2][:, m0:m0 + mm],
                        rhs=g2_sb[k2][:, :], start=(k2 == 0), stop=(k2 == 1))
                nc.vector.tensor_copy(rh_sb[m][:, :], ps[0:mm, :])

            # scatter rh rows into conv1 input blocks
            for b in range(NBLK):
                lo = max(0, 6 * b - 1)
                hi = min(256, 6 * b + 6)
                r0 = lo
                while r0 <= hi:
                    c = r0 // 128
                    c_end = min(hi, c * 128 + 127)
                    cnt = c_end - r0 + 1
                    il0 = r0 - (6 * b - 1)
                    nc.sync.dma_start(
                        x1[b][il0:il0 + cnt, 1:258],
                        rh_sb[c][r0 - c * 128:r0 - c * 128 + cnt, :])
                    r0 = c_end + 1

            # ---------------- conv layers ----------------
            def conv_matmuls(tkey, xin_b, M, b):
                key = tkey + "b" if (b == NBLK - 1 and tkey != "t1") else tkey
                t = tsb[key]
                ps = pp.tile([128, CROP], F32, name="ps", tag="ps")
                for dj in range(3):
                    nc.tensor.matmul(
                        ps[0:M, :], lhsT=t[:, dj * M:(dj + 1) * M],
                        rhs=xin_b[:, dj:dj + CROP],
                        start=(dj == 0), stop=(dj == 2))
                return ps

            def evict_zones(ps, xout, b):
                hi = 96 if b < NBLK - 1 else 80
                # alternate engines per block to double eviction throughput
                if b % 2:
                    nc.scalar.copy(xout[b][0:hi, 1:258], ps[0:hi, :])
                    zeng = nc.vector
                else:
                    nc.vector.tensor_copy(xout[b][0:hi, 1:258], ps[0:hi, :])
                    zeng = nc.scalar
                # il0 of block b+1 (= this block's inn5 dup)
                if b + 1 < NBLK:
                    zeng.copy(xout[b + 1][96:112, 1:258], ps[96:112, :]) \
                        if zeng is nc.scalar else \
                        zeng.tensor_copy(xout[b + 1][96:112, 1:258],
                                         ps[96:112, :])
                # il7 of block b-1 (= this block's inn0 dup, masked; DVE only)
                if b > 0:
                    nc.vector.copy_predicated(xout[b - 1][96:128, 1:258],
                                              mk[96:128, 0:257],
                                              ps[96:128, :])

            def conv_layer(tkey, xin, xout, kin):
                for b in range(NBLK):
                    xi = xin[b][0:kin, :] if kin < 128 else xin[b][:, :]
                    ps = conv_matmuls(tkey, xi, 128, b)
                    evict_zones(ps, xout, b)

            conv_layer("t1", x1, xs["2"], 8)         # conv1
            conv_layer("t2", xs["2"], xs["3"], 128)  # conv2

            # conv3 + theta: xs3 -> xs4 (x4 layout: re [0:48], im [48:96])
            x4 = xs["4"]
            for b in range(NBLK):
                psA = conv_matmuls("t3", xs["3"][b][:, :], 128, b)
                psB = conv_matmuls("t3s", xs["3"][b][:, :], 128, b)
                tht = pw2.tile([128, 2 * CROP], F16, name="tht", tag="tht",
                               bufs=8)
                nc.sync.dma_start(tht[:, :],
                                  dp["thet"][:, b * 2 * CROP:(b + 1) * 2 * CROP])
                m1 = pw2.tile([48, CROP], F32, name="m1", tag="m1")
                m2 = pw2.tile([48, CROP], F32, name="m2", tag="m2")
                m3 = pw2.tile([112, CROP], F32, name="m3", tag="m3")
                m4 = pw2.tile([112, CROP], F32, name="m4", tag="m4")
                # re' = zr*thr - zi*thi  (lanes 0:48: psA=re, psB=im)
                nc.vector.tensor_mul(m1[:, :], psA[0:48, :], tht[0:48, 0:CROP])
                nc.vector.tensor_mul(m2[:, :], psB[0:48, :],
                                     tht[0:48, CROP:2 * CROP])
                nc.vector.tensor_sub(x4[b][0:48, 1:258], m1[:, :], m2[:, :])
                # im' = zi*thr + zr*thi (lanes 64:112: psA=im, psB=re)
                nc.vector.tensor_mul(m3[64:112, :], psA[64:112, :],
                                     tht[64:112, 0:CROP])
                nc.vector.tensor_mul(m4[64:112, :], psB[64:112, :],
                                     tht[64:112, CROP:2 * CROP])
                nc.vector.tensor_add(x4[b][64:112, 1:258], m3[64:112, :],
                                     m4[64:112, :])
                # halos via small DMAs (x4 layout is lane-incompatible)
                if b > 0:
                    nc.sync.dma_start(x4[b][48:56, 1:258],
                                      x4[b - 1][40:48, 1:258])
                    nc.sync.dma_start(x4[b][56:64, 1:258],
                                      x4[b - 1][104:112, 1:258])
                    nc.sync.dma_start(x4[b - 1][112:120, 1:258],
                                      x4[b][0:8, 1:258])
                    nc.sync.dma_start(x4[b - 1][120:128, 1:258],
                                      x4[b][64:72, 1:258])

            conv_layer("t4", xs["4"], xs["5"], 128)  # conv4
            conv_layer("t5", xs["5"], xs["6"], 128)  # conv5

            # conv6: xs6 -> Xout planes [257, 257] re/im (3 chunks each)
            xo = {}
            for p in ("re", "im"):
                xo[p] = [pw.tile([128, CROP], F16, name=f"xo{p}0", tag=f"xo{p}0"),
                         pw.tile([128, CROP], F16, name=f"xo{p}1", tag=f"xo{p}1"),
                         pw.tile([1, CROP], F16, name=f"xo{p}2", tag=f"xo{p}2")]
            for b in range(NBLK):
                ps = conv_matmuls("t6", xs["6"][b][:, :], 12, b)
                y6 = pw2.tile([12, CROP], F16, name="y6", tag="y6", bufs=4)
                nc.scalar.copy(y6[:, :], ps[0:12, :])
                nil = 6 if b < NBLK - 1 else 5
                for pi, p in enumerate(("re", "im")):
                    r0 = 6 * b
                    while r0 < 6 * b + nil:
                        c = r0 // 128
                        c_end = min(6 * b + nil - 1, c * 128 + 127)
                        cnt = c_end - r0 + 1
                        nc.sync.dma_start(
                            xo[p][c][r0 - c * 128:r0 - c * 128 + cnt, :],
                            y6[pi * 6 + (r0 - 6 * b):pi * 6 + (r0 - 6 * b) + cnt, :])
                        r0 = c_end + 1

            # ---------------- back transform ----------------
            at = {}
            for p in ("re", "im"):
                at[p] = [pw.tile([128, N1], F16, name=f"at{p}0", tag=f"at{p}0"),
                         pw.tile([128, N1], F16, name=f"at{p}1", tag=f"at{p}1"),
                         pw.tile([1, N1], F16, name=f"at{p}2", tag=f"at{p}2")]
            for m, (m0, mm) in enumerate(((0, 128), (128, 128), (256, 1))):
                for p, terms in (("re", (("re", hr_sb), ("im", hn_sb))),
                                 ("im", (("re", hi_sb), ("im", hr_sb)))):
                    ps = pp.tile([128, N1], F32, name="ps", tag="ps")
                    nmm = 0
                    for (xp, hsb) in terms:
                        for k2 in range(3):
                            nc.tensor.matmul(
                                ps[0:mm, :],
                                lhsT=xo[xp][k2][:, m0:m0 + mm],
                                rhs=hsb[k2][:, :],
                                start=(nmm == 0), stop=(nmm == 5))
                            nmm += 1
                    nc.scalar.copy(at[p][m][:, :], ps[0:mm, :])

            e_sb = {}
            for p in ("re", "im"):
                e_sb[p] = [pw.tile([128, N1], F32, name=f"e{p}0", tag=f"e{p}0"),
                           pw.tile([127, N1], F32, name=f"e{p}1", tag=f"e{p}1")]
            for m, (m0, mm) in enumerate(((0, 128), (128, 127))):
                for p, terms in (("re", (("re", hr_sb), ("im", hn_sb))),
                                 ("im", (("re", hi_sb), ("im", hr_sb)))):
                    ps = pp.tile([128, N1], F32, name="ps", tag="ps")
                    nmm = 0
                    for (ap_, hsb) in terms:
                        for k2 in range(3):
                            nc.tensor.matmul(
                                ps[0:mm, :],
                                lhsT=at[ap_][k2][:, m0:m0 + mm],
                                rhs=hsb[k2][:, :],
                                start=(nmm == 0), stop=(nmm == 5))
                            nmm += 1
                    nc.vector.tensor_copy(e_sb[p][m][:, :], ps[0:mm, :])

            for p, dram in (("re", ere), ("im", eim)):
                nc.sync.dma_start(dram[0:128, :], e_sb[p][0][:, :])
                nc.sync.dma_start(dram[128:255, :], e_sb[p][1][:, :])

    nc.finalize()
    return nc


_NC_CACHE = None


def _get_nc():
    global _NC_CACHE
    if _NC_CACHE is None:
        _NC_CACHE = _build_nc()
    return _NC_CACHE


def kernel(**inputs):
    global LAST_EXEC_TIME_NS
    inputs = {k: np.asarray(v) for k, v in inputs.items()}
    consts = _host_consts()
    in_maps = [_host_prep_sample(b, inputs, consts) for b in range(B)]
    nc = _get_nc()
    trace = bool(os.environ.get("BASS_TRACE"))
    res = run_bass_kernel_spmd(nc, in_maps, list(range(B)), trace=trace)
    LAST_EXEC_TIME_NS = res.exec_time_ns
    out = np.zeros((B, 1, N1, N1), np.complex64)
    for b in range(B):
        out[b, 0] = res.results[b]["ere"] + 1j * res.results[b]["eim"]
    return out


# revision 17
# speedup vs baseline: 1.0558x; 1.0156x over previous
"""FNS spectral network kernel for 8x TRN2 NeuronCores (data parallel over batch).

Math (verified vs reference to ~3e-7 in fp64):
  per sample b:
    rh = (-Gi) @ r @ Gi.T          Gi[j,n] = sin(pi*(j-128)*(n+1)/256)/256, [257,255]
    x  = conv1(rh) -> conv2 -> conv3 -> *theta -> conv4 -> conv5 -> conv6
         (3x3 per-sample complex convs, pad=1; conv4..6 use _wT weights)
    e  = H @ x @ H.T               H[k,j] = exp(-2i*pi*k*(j-127)/513), [255,257]

Device mapping (1 sample/core, all weights/transforms host-preprocessed):
  - convs as block-Toeplitz matmuls: image in 43 row-blocks of 8 rows
    (stride 6, 1-row halo each side). Std layout: mid rows il=1..6 at
    partitions (il-1)*16+cp in [0:96] (cp=reim*8+ch), il0 halo at [96:112],
    il7 at [112:128]. Free = 259 cols (zero pad col each side). Per block:
    3 matmuls (dj taps via free-dim shifts), stationary T [128, 3*M].
  - T has 32 duplicated output columns ([96:112]=inn5 dup, [112:128]=inn0
    dup) so halo propagation is done by lane-aligned compute copies from
    PSUM (plain copy for il0, copy_predicated for il7) - no partition-
    shifting DMAs in the conv pipeline.
  - theta: conv3 emits (reim,inn,ch) order; DVE complex-multiplies write
    straight into conv4's input tiles (x4 uses a (reim,il,ch) layout);
    only zi lane-move + 4 small halo DMAs per block remain.
  - block 42 uses T variants with rows for image rows 257/258 zeroed, so
    out-of-image garbage partitions are never observed.
  - front/back transforms: dense matmuls using the (A@B)^T = matmul(lhsT=A,
    rhs=B) identity so no on-device transposes are needed.
  - fp16 operands (validated ~1e-3 rel err), fp32 PSUM accumulate.
  - Bacc (not raw Bass) so multi-wait instructions are legalized for TRN2.
"""

import os

import numpy as np

import concourse.bacc as bacc
import concourse.mybir as mybir
from concourse.bass_utils import run_bass_kernel_spmd
from concourse.tile import TileContext

F16 = mybir.dt.float16
F32 = mybir.dt.float32
U8 = mybir.dt.uint8

B = 8
N1 = 255
CROP = 257
CH = 8
NBLK = 43          # ceil(257/6)
WPAD = 259         # 257 cols + 1 zero col each side

LAST_EXEC_TIME_NS = None


# ----------------------------------------------------------------------------
# Host-side constant / weight preprocessing
# ----------------------------------------------------------------------------

def _host_consts():
    j = np.arange(CROP)[:, None]
    n = np.arange(N1)[None, :]
    Gi = (np.sin(np.pi * (j - 128) * (n + 1) / 256.0) / 256.0).astype(np.float32)
    k = np.arange(N1)[:, None]
    jj = np.arange(CROP)[None, :]
    H = np.exp(-2j * np.pi * k * (jj - 127.0) / 513.0)
    return {
        "g1t": np.ascontiguousarray((-Gi).T.astype(np.float16)),   # [255,257]
        "g2t": np.ascontiguousarray(Gi.T.astype(np.float16)),      # [255,257]
        "hrt": np.ascontiguousarray(H.real.T.astype(np.float16)),  # [257,255]
        "hit": np.ascontiguousarray(H.imag.T.astype(np.float16)),  # [257,255]
        "hnit": np.ascontiguousarray((-H.imag).T.astype(np.float16)),
    }


def _expand_w(wre, wim):
    """[Co,Ci,3,3] complex -> real packed [2Co, 2Ci, 3, 3], part = reim*C+ch."""
    Co, Ci = wre.shape[0], wre.shape[1]
    W = np.zeros((2 * Co, 2 * Ci, 3, 3), np.float32)
    W[:Co, :Ci] = wre
    W[:Co, Ci:] = -wim
    W[Co:, :Ci] = wim
    W[Co:, Ci:] = wre
    return W


def _wT(wre, wim):
    """torch _wT: swap cout/cin, transpose 3x3 kernel, conjugate."""
    wre2 = np.swapaxes(np.swapaxes(wre, 0, 1), -2, -1)
    wim2 = -np.swapaxes(np.swapaxes(wim, 0, 1), -2, -1)
    return wre2, wim2


def _row_std(p):
    if p < 96:
        return 1 + p // 16, p % 16
    if p < 112:
        return 0, p - 96
    return 7, p - 112


def _row_x4(p):
    """x4 (theta output) layout: re mid [0:48], il0 halos [48:64] (re,im),
    im mid [64:112], il7 halos [112:128] (re,im)."""
    if p < 48:
        return 1 + p // 8, p % 8
    if p < 64:
        return 0, p - 48         # [48:56] re ch, [56:64] im ch
    if p < 112:
        q = p - 64
        return 1 + q // 8, 8 + q % 8
    return 7, p - 112            # [112:120] re ch, [120:128] im ch


def _col_std_dup(m):
    """M=128 col map with dup halo outputs: [0:96] mid, [96:112] inn5 dup,
    [112:128] inn0 dup."""
    if m < 96:
        return m // 16, m % 16
    if m < 112:
        return 5, m - 96
    return 0, m - 112


def _build_T(Wexp, rowmap, colmap, K, M, zero42=False):
    T = np.zeros((K, 3 * M), np.float32)
    Cin2 = Wexp.shape[1]
    for p in range(K):
        il, cp = rowmap(p)
        if cp >= Cin2:
            continue
        if zero42 and il >= 6:
            continue
        for dj in range(3):
            for m in range(M):
                co = colmap(m)
                if co is None:
                    continue
                inn, op = co
                di = il - inn
                if 0 <= di <= 2:
                    T[p, dj * M + m] = Wexp[op, cp, di, dj]
    return T.astype(np.float16)


def _host_prep_sample(bidx, inputs, consts):
    s = {}
    s["r16"] = np.ascontiguousarray(inputs["r"][bidx, 0].astype(np.float16))
    s.update(consts)

    w1 = (inputs["w1_re"][bidx], inputs["w1_im"][bidx])  # [8,1,3,3]
    w2 = (inputs["w2_re"][bidx], inputs["w2_im"][bidx])
    w3 = (inputs["w3_re"][bidx], inputs["w3_im"][bidx])

    W1r = _expand_w(*w1)[:, 0:1]          # [16, 1, 3, 3] (input is real rh)
    W2 = _expand_w(*w2)
    W3 = _expand_w(*w3)
    W4 = _expand_w(*_wT(*w3))
    W5 = _expand_w(*_wT(*w2))
    W6 = _expand_w(*_wT(*w1))             # [2, 16, 3, 3]

    def col_c3a(m):
        if m < 48:
            return m // 8, m % 8          # re
        if 64 <= m < 112:
            q = m - 64
            return q // 8, 8 + q % 8      # im
        return None

    def col_c3b(m):
        if m < 48:
            return m // 8, 8 + m % 8      # im
        if 64 <= m < 112:
            q = m - 64
            return q // 8, q % 8          # re
        return None

    def col_c6(m):
        return m % 6, m // 6

    def row_x1(p):
        return p, 0

    s["t1"] = _build_T(W1r, row_x1, _col_std_dup, 8, 128)
    s["t2"] = _build_T(W2, _row_std, _col_std_dup, 128, 128)
    s["t2b"] = _build_T(W2, _row_std, _col_std_dup, 128, 128, zero42=True)
    s["t3"] = _build_T(W3, _row_std, col_c3a, 128, 128)
    s["t3b"] = _build_T(W3, _row_std, col_c3a, 128, 128, zero42=True)
    s["t3s"] = _build_T(W3, _row_std, col_c3b, 128, 128)
    s["t3sb"] = _build_T(W3, _row_std, col_c3b, 128, 128, zero42=True)
    s["t4"] = _build_T(W4, _row_x4, _col_std_dup, 128, 128)
    s["t4b"] = _build_T(W4, _row_x4, _col_std_dup, 128, 128, zero42=True)
    s["t5"] = _build_T(W5, _row_std, _col_std_dup, 128, 128)
    s["t5b"] = _build_T(W5, _row_std, _col_std_dup, 128, 128, zero42=True)
    s["t6"] = _build_T(W6, _row_std, col_c6, 128, 12)
    s["t6b"] = _build_T(W6, _row_std, col_c6, 128, 12, zero42=True)

    # theta per-block tiles: [48, NBLK * 2*257], block b re at +0, im at +257;
    # row = inn*8+ch matching conv3's (reim,inn,ch) output order
    th = np.zeros((128, NBLK * 2 * CROP), np.float16)
    tr = inputs["theta_re"][bidx]  # [8, 257, 257]
    ti = inputs["theta_im"][bidx]
    for b in range(NBLK):
        ninn = 6 if b < NBLK - 1 else 5
        base = b * 2 * CROP
        for inn in range(ninn):
            row = 6 * b + inn
            for ch in range(CH):
                p = inn * 8 + ch
                th[p, base:base + CROP] = tr[ch, row]
                th[64 + p, base:base + CROP] = tr[ch, row]
                th[p, base + CROP:base + 2 * CROP] = -ti[ch, row]
                th[64 + p, base + CROP:base + 2 * CROP] = ti[ch, row]
    s["thet"] = th
    return s


# ----------------------------------------------------------------------------
# Device program
# ----------------------------------------------------------------------------

def _build_nc():
    nc = bacc.Bacc(None, target_bir_lowering=False, debug=False)

    dp = {}
    for name, shape, dt in (
        ("r16", [N1, N1], F16), ("g1t", [N1, CROP], F16),
        ("g2t", [N1, CROP], F16), ("hrt", [CROP, N1], F16),
        ("hit", [CROP, N1], F16), ("hnit", [CROP, N1], F16),
        ("t1", [8, 384], F16), ("t2", [128, 384], F16),
        ("t2b", [128, 384], F16), ("t3", [128, 384], F16),
        ("t3b", [128, 384], F16),
        ("t3s", [128, 384], F16), ("t3sb", [128, 384], F16), ("t4", [128, 384], F16),
        ("t4b", [128, 384], F16), ("t5", [128, 384], F16),
        ("t5b", [128, 384], F16), ("t6", [128, 36], F16),
        ("t6b", [128, 36], F16), ("thet", [128, NBLK * 2 * CROP], F16),
    ):
        dp[name] = nc.declare_dram_parameter(name, list(shape), dt,
                                             isOutput=False)
    ere = nc.declare_dram_parameter("ere", [N1, N1], F32, isOutput=True)
    eim = nc.declare_dram_parameter("eim", [N1, N1], F32, isOutput=True)

    with TileContext(nc) as tc:
        with (
            tc.tile_pool(name="const", bufs=1) as pc,
            tc.tile_pool(name="xbuf", bufs=1) as px,
            tc.tile_pool(name="work", bufs=1) as pw,
            tc.tile_pool(name="wk2", bufs=3) as pw2,
            tc.tile_pool(name="psum", bufs=8, space="PSUM") as pp,
        ):
            # ---------------- constant loads ----------------
            def load_const(name):
                shape = [int(x) for x in dp[name].shape]
                t = pc.tile(shape, F16, name=name, tag=name)
                nc.sync.dma_start(t[:, :], dp[name][:, :])
                return t

            def load_chunks(name, rows, cols):
                out = []
                r0 = 0
                while r0 < rows:
                    rr = min(128, rows - r0)
                    t = pc.tile([rr, cols], F16, name=f"{name}{r0}",
                                tag=f"{name}{r0}")
                    nc.sync.dma_start(t[:, :], dp[name][r0:r0 + rr, :])
                    out.append(t)
                    r0 += rr
                return out

            r_sb = load_chunks("r16", N1, N1)            # [128,255],[127,255]
            g1_sb = load_chunks("g1t", N1, CROP)
            g2_sb = load_chunks("g2t", N1, CROP)
            hr_sb = load_chunks("hrt", CROP, N1)         # [128],[128],[1]
            hi_sb = load_chunks("hit", CROP, N1)
            hn_sb = load_chunks("hnit", CROP, N1)
            tsb = {k: load_const(k) for k in
                   ("t1", "t2", "t2b", "t3", "t3b", "t4", "t4b",
                    "t5", "t5b", "t6", "t6b", "t3s", "t3sb")}

            # ---------------- X block buffers (write-once) ----------------
            x1 = [px.tile([8, WPAD], F16, name=f"x1_{b}", tag=f"x1_{b}")
                  for b in range(NBLK)]
            xs = {}
            for li in ("2", "3", "4", "5", "6"):
                xs[li] = [px.tile([128, WPAD], F16, name=f"x{li}_{b}",
                                  tag=f"x{li}_{b}") for b in range(NBLK)]

            # predication mask for il7-halo copies: 1 on [112:128], 0 on [96:112]
            mk = pw.tile([128, CROP], U8, name="mk", tag="mk")
            nc.gpsimd.memset(mk[96:128, :], 1.0)
            nc.gpsimd.memset(mk[96:112, :], 0.0)

            for b in range(NBLK):
                nc.vector.memset(x1[b][:, :], 0.0)
            for li, tiles in xs.items():
                for b in range(NBLK):
                    nc.gpsimd.memset(tiles[b][:, 0:259:258], 0.0)
                if li == "4":
                    # il0 zones live at [48:64]; start-48 is illegal so clear
                    # [32:64] (the [32:48] part is later overwritten by theta)
                    nc.gpsimd.memset(tiles[0][32:64, :], 0.0)
                else:
                    nc.gpsimd.memset(tiles[0][96:112, :], 0.0)  # il0 of blk 0
                # block 42: il6/il7 zones are never written (T*b zeroes their
                # weights); full memset keeps reads initialized
                nc.vector.memset(tiles[NBLK - 1][:, :], 0.0)

            # ---------------- front transform ----------------
            # Vt = r^T @ G1^T = (G1 r)^T   [255, 257]
            vt_sb = [pw.tile([128, CROP], F16, name="vt0", tag="vt0"),
                     pw.tile([127, CROP], F16, name="vt1", tag="vt1")]
            for m, (m0, mm) in enumerate(((0, 128), (128, 127))):
                ps = pp.tile([128, CROP], F32, name="ps", tag="ps")
                for k2 in range(2):
                    nc.tensor.matmul(
                        ps[0:mm, :], lhsT=r_sb[k2][:, m0:m0 + mm],
                        rhs=g1_sb[k2][:, :], start=(k2 == 0), stop=(k2 == 1))
                nc.scalar.copy(vt_sb[m][:, :], ps[0:mm, :])

            # rh = Vt^T @ G2^T = G1 r G2^T   [257, 257]
            rh_sb = [pw.tile([128, CROP], F16, name="rh0", tag="rh0"),
                     pw.tile([128, CROP], F16, name="rh1", tag="rh1"),
                     pw.tile([1, CROP], F16, name="rh2", tag="rh2")]
            for m, (m0, mm) in enumerate(((0, 128), (128, 128), (256, 1))):
                ps = pp.tile([128, CROP], F32, name="ps", tag="ps")
                for k2 in range(2):
                    nc.tensor.matmul(
                        ps[0:mm, :], lhsT=vt_sb[k2][:, m0:m0 + mm],
                        rhs=g2_sb[k2][:, :], start=(k2 == 0), stop=(k2 == 1))
                nc.vector.tensor_copy(rh_sb[m][:, :], ps[0:mm, :])

            # scatter rh rows into conv1 input blocks
            for b in range(NBLK):
                lo = max(0, 6 * b - 1)
                hi = min(256, 6 * b + 6)
                r0 = lo
                while r0 <= hi:
                    c = r0 // 128
                    c_end = min(hi, c * 128 + 127)
                    cnt = c_end - r0 + 1
                    il0 = r0 - (6 * b - 1)
                    nc.sync.dma_start(
                        x1[b][il0:il0 + cnt, 1:258],
                        rh_sb[c][r0 - c * 128:r0 - c * 128 + cnt, :])
                    r0 = c_end + 1

            # ---------------- conv layers ----------------
            def conv_matmuls(tkey, xin_b, M, b):
                key = tkey + "b" if (b == NBLK - 1 and tkey != "t1") else tkey
                t = tsb[key]
                ps = pp.tile([128, CROP], F32, name="ps", tag="ps")
                for dj in range(3):
                    nc.tensor.matmul(
                        ps[0:M, :], lhsT=t[:, dj * M:(dj + 1) * M],
                        rhs=xin_b[:, dj:dj + CROP],
                        start=(dj == 0), stop=(dj == 2))
                return ps

            def evict_zones(ps, xout, b):
                hi = 96 if b < NBLK - 1 else 80
                # alternate engines per block to double eviction throughput
                if b % 2:
                    nc.scalar.copy(xout[b][0:hi, 1:258], ps[0:hi, :])
                    zeng = nc.vector
                else:
                    nc.vector.tensor_copy(xout[b][0:hi, 1:258], ps[0:hi, :])
                    zeng = nc.scalar
                # il0 of block b+1 (= this block's inn5 dup)
                if b + 1 < NBLK:
                    zeng.copy(xout[b + 1][96:112, 1:258], ps[96:112, :]) \
                        if zeng is nc.scalar else \
                        zeng.tensor_copy(xout[b + 1][96:112, 1:258],
                                         ps[96:112, :])
                # il7 of block b-1 (= this block's inn0 dup, masked; DVE only)
                if b > 0:
                    nc.vector.copy_predicated(xout[b - 1][96:128, 1:258],
                                              mk[96:128, 0:257],
                                              ps[96:128, :])

            def conv_layer(tkey, xin, xout, kin):
                for b in range(NBLK):
                    xi = xin[b][0:kin, :] if kin < 128 else xin[b][:, :]
                    ps = conv_matmuls(tkey, xi, 128, b)
                    evict_zones(ps, xout, b)

            conv_layer("t1", x1, xs["2"], 8)         # conv1
            conv_layer("t2", xs["2"], xs["3"], 128)  # conv2

            # conv3 + theta: xs3 -> xs4 (x4 layout: re [0:48], im [48:96])
            x4 = xs["4"]
            for b in range(NBLK):
                psA = conv_matmuls("t3", xs["3"][b][:, :], 128, b)
                psB = conv_matmuls("t3s", xs["3"][b][:, :], 128, b)
                tht = pw2.tile([128, 2 * CROP], F16, name="tht", tag="tht",
                               bufs=8)
                nc.gpsimd.dma_start(tht[:, :],
                                    dp["thet"][:, b * 2 * CROP:(b + 1) * 2 * CROP])
                # u = psA*thx, v = psB*thy over lanes [0:112] in one pass;
                # signs are baked into thy so re' and im' are both u+v
                u = pw2.tile([112, CROP], F32, name="m1", tag="m1")
                v = pw2.tile([112, CROP], F32, name="m2", tag="m2")
                nc.vector.tensor_mul(u[:, :], psA[0:112, :],
                                     tht[0:112, 0:CROP])
                nc.vector.tensor_mul(v[:, :], psB[0:112, :],
                                     tht[0:112, CROP:2 * CROP])
                nc.vector.tensor_add(x4[b][0:48, 1:258], u[0:48, :], v[0:48, :])
                nc.vector.tensor_add(x4[b][64:112, 1:258], u[64:112, :],
                                     v[64:112, :])
                # halos via small DMAs (x4 layout is lane-incompatible)
                if b > 0:
                    nc.gpsimd.dma_start(x4[b][48:56, 1:258],
                                        x4[b - 1][40:48, 1:258])
                    nc.gpsimd.dma_start(x4[b][56:64, 1:258],
                                        x4[b - 1][104:112, 1:258])
                    nc.gpsimd.dma_start(x4[b - 1][112:120, 1:258],
                                        x4[b][0:8, 1:258])
                    nc.gpsimd.dma_start(x4[b - 1][120:128, 1:258],
                                        x4[b][64:72, 1:258])

            conv_layer("t4", xs["4"], xs["5"], 128)  # conv4
            conv_layer("t5", xs["5"], xs["6"], 128)  # conv5

            # conv6: xs6 -> Xout planes [257, 257] re/im (3 chunks each)
            xo = {}
            for p in ("re", "im"):
                xo[p] = [pw.tile([128, CROP], F16, name=f"xo{p}0", tag=f"xo{p}0"),
                         pw.tile([128, CROP], F16, name=f"xo{p}1", tag=f"xo{p}1"),
                         pw.tile([1, CROP], F16, name=f"xo{p}2", tag=f"xo{p}2")]
            for b in range(NBLK):
                ps = conv_matmuls("t6", xs["6"][b][:, :], 12, b)
                y6 = pw2.tile([12, CROP], F16, name="y6", tag="y6", bufs=4)
                nc.scalar.copy(y6[:, :], ps[0:12, :])
                nil = 6 if b < NBLK - 1 else 5
                for pi, p in enumerate(("re", "im")):
                    r0 = 6 * b
                    while r0 < 6 * b + nil:
                        c = r0 // 128
                        c_end = min(6 * b + nil - 1, c * 128 + 127)
                        cnt = c_end - r0 + 1
                        nc.sync.dma_start(
                            xo[p][c][r0 - c * 128:r0 - c * 128 + cnt, :],
                            y6[pi * 6 + (r0 - 6 * b):pi * 6 + (r0 - 6 * b) + cnt, :])
                        r0 = c_end + 1

            # ---------------- back transform ----------------
            at = {}
            for p in ("re", "im"):
                at[p] = [pw.tile([128, N1], F16, name=f"at{p}0", tag=f"at{p}0"),
                         pw.tile([128, N1], F16, name=f"at{p}1", tag=f"at{p}1"),
                         pw.tile([1, N1], F16, name=f"at{p}2", tag=f"at{p}2")]
            for m, (m0, mm) in enumerate(((0, 128), (128, 128), (256, 1))):
                for p, terms in (("re", (("re", hr_sb), ("im", hn_sb))),
                                 ("im", (("re", hi_sb), ("im", hr_sb)))):
                    ps = pp.tile([128, N1], F32, name="ps", tag="ps")
                    nmm = 0
                    for (xp, hsb) in terms:
                        for k2 in range(3):
                            nc.tensor.matmul(
                                ps[0:mm, :],
                                lhsT=xo[xp][k2][:, m0:m0 + mm],
                                rhs=hsb[k2][:, :],
                                start=(nmm == 0), stop=(nmm == 5))
                            nmm += 1
                    nc.scalar.copy(at[p][m][:, :], ps[0:mm, :])

            e_sb = {}
            for p in ("re", "im"):
                e_sb[p] = [pw.tile([128, N1], F32, name=f"e{p}0", tag=f"e{p}0"),
                           pw.tile([127, N1], F32, name=f"e{p}1", tag=f"e{p}1")]
            for m, (m0, mm) in enumerate(((0, 128), (128, 127))):
                for p, terms in (("re", (("re", hr_sb), ("im", hn_sb))),
                                 ("im", (("re", hi_sb), ("im", hr_sb)))):
                    ps = pp.tile([128, N1], F32, name="ps", tag="ps")
                    nmm = 0
                    for (ap_, hsb) in terms:
                        for k2 in range(3):
                            nc.tensor.matmul(
                                ps[0:mm, :],
                                lhsT=at[ap_][k2][:, m0:m0 + mm],
                                rhs=hsb[k2][:, :],
                                start=(nmm == 0), stop=(nmm == 5))
                            nmm += 1
                    nc.vector.tensor_copy(e_sb[p][m][:, :], ps[0:mm, :])

            for p, dram in (("re", ere), ("im", eim)):
                nc.sync.dma_start(dram[0:128, :], e_sb[p][0][:, :])
                nc.sync.dma_start(dram[128:255, :], e_sb[p][1][:, :])

    nc.finalize()
    return nc


_NC_CACHE = None


def _get_nc():
    global _NC_CACHE
    if _NC_CACHE is None:
        _NC_CACHE = _build_nc()
    return _NC_CACHE


def kernel(**inputs):
    global LAST_EXEC_TIME_NS
    inputs = {k: np.asarray(v) for k, v in inputs.items()}
    consts = _host_consts()
    in_maps = [_host_prep_sample(b, inputs, consts) for b in range(B)]
    nc = _get_nc()
    trace = bool(os.environ.get("BASS_TRACE"))
    res = run_bass_kernel_spmd(nc, in_maps, list(range(B)), trace=trace)
    LAST_EXEC_TIME_NS = res.exec_time_ns
    out = np.zeros((B, 1, N1, N1), np.complex64)
    for b in range(B):
        out[b, 0] = res.results[b]["ere"] + 1j * res.results[b]["eim"]
    return out


# revision 18
# speedup vs baseline: 1.2101x; 1.1461x over previous
"""FNS spectral network kernel for 8x TRN2 NeuronCores (data parallel over batch).

Math (verified vs reference to ~3e-7 in fp64):
  per sample b:
    rh = (-Gi) @ r @ Gi.T          Gi[j,n] = sin(pi*(j-128)*(n+1)/256)/256, [257,255]
    x  = conv1(rh) -> conv2 -> conv3 -> *theta -> conv4 -> conv5 -> conv6
         (3x3 per-sample complex convs, pad=1; conv4..6 use _wT weights)
    e  = H @ x @ H.T               H[k,j] = exp(-2i*pi*k*(j-127)/513), [255,257]

Device mapping (1 sample/core, all weights/transforms host-preprocessed):
  - convs as block-Toeplitz matmuls: image in 43 row-blocks of 8 rows
    (stride 6, 1-row halo each side). Std layout: mid rows il=1..6 at
    partitions (il-1)*16+cp in [0:96] (cp=reim*8+ch), il0 halo at [96:112],
    il7 at [112:128]. Free = 259 cols (zero pad col each side). Per block:
    3 matmuls (dj taps via free-dim shifts), stationary T [128, 3*M].
  - T has 32 duplicated output columns ([96:112]=inn5 dup, [112:128]=inn0
    dup) so halo propagation is done by lane-aligned compute copies from
    PSUM (plain copy for il0, copy_predicated for il7) - no partition-
    shifting DMAs in the conv pipeline.
  - theta: conv3 emits (reim,inn,ch) order; DVE complex-multiplies write
    straight into conv4's input tiles (x4 uses a (reim,il,ch) layout);
    only zi lane-move + 4 small halo DMAs per block remain.
  - block 42 uses T variants with rows for image rows 257/258 zeroed, so
    out-of-image garbage partitions are never observed.
  - front/back transforms: dense matmuls using the (A@B)^T = matmul(lhsT=A,
    rhs=B) identity so no on-device transposes are needed.
  - fp16 operands (validated ~1e-3 rel err), fp32 PSUM accumulate.
  - Bacc (not raw Bass) so multi-wait instructions are legalized for TRN2.
"""

import os

import numpy as np

import concourse.bacc as bacc
import concourse.mybir as mybir
from concourse.bass_utils import run_bass_kernel_spmd
from concourse.tile import TileContext

F16 = mybir.dt.float16
F32 = mybir.dt.float32
U8 = mybir.dt.uint8

B = 8
N1 = 255
CROP = 257
CH = 8
NBLK = 43          # ceil(257/6)
WPAD = 259         # 257 cols + 1 zero col each side

LAST_EXEC_TIME_NS = None


# ----------------------------------------------------------------------------
# Host-side constant / weight preprocessing
# ----------------------------------------------------------------------------

def _host_consts():
    j = np.arange(CROP)[:, None]
    n = np.arange(N1)[None, :]
    Gi = (np.sin(np.pi * (j - 128) * (n + 1) / 256.0) / 256.0).astype(np.float32)
    k = np.arange(N1)[:, None]
    jj = np.arange(CROP)[None, :]
    H = np.exp(-2j * np.pi * k * (jj - 127.0) / 513.0)
    return {
        "g1t": np.ascontiguousarray((-Gi).T.astype(np.float16)),   # [255,257]
        "g2t": np.ascontiguousarray(Gi.T.astype(np.float16)),      # [255,257]
        "hrt": np.ascontiguousarray(H.real.T.astype(np.float16)),  # [257,255]
        "hit": np.ascontiguousarray(H.imag.T.astype(np.float16)),  # [257,255]
        "hnit": np.ascontiguousarray((-H.imag).T.astype(np.float16)),
    }


def _expand_w(wre, wim):
    """[Co,Ci,3,3] complex -> real packed [2Co, 2Ci, 3, 3], part = reim*C+ch."""
    Co, Ci = wre.shape[0], wre.shape[1]
    W = np.zeros((2 * Co, 2 * Ci, 3, 3), np.float32)
    W[:Co, :Ci] = wre
    W[:Co, Ci:] = -wim
    W[Co:, :Ci] = wim
    W[Co:, Ci:] = wre
    return W


def _wT(wre, wim):
    """torch _wT: swap cout/cin, transpose 3x3 kernel, conjugate."""
    wre2 = np.swapaxes(np.swapaxes(wre, 0, 1), -2, -1)
    wim2 = -np.swapaxes(np.swapaxes(wim, 0, 1), -2, -1)
    return wre2, wim2


def _row_std(p):
    if p < 96:
        return 1 + p // 16, p % 16
    if p < 112:
        return 0, p - 96
    return 7, p - 112


def _row_x4(p):
    """x4 (theta output) layout: re mid [0:48], il0 halos [48:64] (re,im),
    im mid [64:112], il7 halos [112:128] (re,im)."""
    if p < 48:
        return 1 + p // 8, p % 8
    if p < 64:
        return 0, p - 48         # [48:56] re ch, [56:64] im ch
    if p < 112:
        q = p - 64
        return 1 + q // 8, 8 + q % 8
    return 7, p - 112            # [112:120] re ch, [120:128] im ch


def _col_std_dup(m):
    """M=128 col map with dup halo outputs: [0:96] mid, [96:112] inn5 dup,
    [112:128] inn0 dup."""
    if m < 96:
        return m // 16, m % 16
    if m < 112:
        return 5, m - 96
    return 0, m - 112


def _build_T(Wexp, rowmap, colmap, K, M, zero42=False):
    T = np.zeros((K, 3 * M), np.float32)
    Cin2 = Wexp.shape[1]
    for p in range(K):
        il, cp = rowmap(p)
        if cp >= Cin2:
            continue
        if zero42 and il >= 6:
            continue
        for dj in range(3):
            for m in range(M):
                co = colmap(m)
                if co is None:
                    continue
                inn, op = co
                di = il - inn
                if 0 <= di <= 2:
                    T[p, dj * M + m] = Wexp[op, cp, di, dj]
    return T.astype(np.float16)


def _host_prep_sample(bidx, inputs, consts):
    s = {}
    s["r16"] = np.ascontiguousarray(inputs["r"][bidx, 0].astype(np.float16))
    s.update(consts)

    w1 = (inputs["w1_re"][bidx], inputs["w1_im"][bidx])  # [8,1,3,3]
    w2 = (inputs["w2_re"][bidx], inputs["w2_im"][bidx])
    w3 = (inputs["w3_re"][bidx], inputs["w3_im"][bidx])

    W1r = _expand_w(*w1)[:, 0:1]          # [16, 1, 3, 3] (input is real rh)
    W2 = _expand_w(*w2)
    W3 = _expand_w(*w3)
    W4 = _expand_w(*_wT(*w3))
    W5 = _expand_w(*_wT(*w2))
    W6 = _expand_w(*_wT(*w1))             # [2, 16, 3, 3]

    def col_c3a(m):
        if m < 48:
            return m // 8, m % 8          # re
        if 64 <= m < 112:
            q = m - 64
            return q // 8, 8 + q % 8      # im
        return None

    def col_c3b(m):
        if m < 48:
            return m // 8, 8 + m % 8      # im
        if 64 <= m < 112:
            q = m - 64
            return q // 8, q % 8          # re
        return None

    def col_c6(m):
        return m % 6, m // 6

    def row_x1(p):
        return p, 0

    s["t1"] = _build_T(W1r, row_x1, _col_std_dup, 8, 128)
    s["t2"] = _build_T(W2, _row_std, _col_std_dup, 128, 128)
    s["t2b"] = _build_T(W2, _row_std, _col_std_dup, 128, 128, zero42=True)
    s["t3"] = _build_T(W3, _row_std, col_c3a, 128, 128)
    s["t3b"] = _build_T(W3, _row_std, col_c3a, 128, 128, zero42=True)
    s["t3s"] = _build_T(W3, _row_std, col_c3b, 128, 128)
    s["t3sb"] = _build_T(W3, _row_std, col_c3b, 128, 128, zero42=True)
    s["t4"] = _build_T(W4, _row_x4, _col_std_dup, 128, 128)
    s["t4b"] = _build_T(W4, _row_x4, _col_std_dup, 128, 128, zero42=True)
    s["t5"] = _build_T(W5, _row_std, _col_std_dup, 128, 128)
    s["t5b"] = _build_T(W5, _row_std, _col_std_dup, 128, 128, zero42=True)
    s["t6"] = _build_T(W6, _row_std, col_c6, 128, 12)
    s["t6b"] = _build_T(W6, _row_std, col_c6, 128, 12, zero42=True)

    # theta per-block tiles: [48, NBLK * 2*257], block b re at +0, im at +257;
    # row = inn*8+ch matching conv3's (reim,inn,ch) output order
    th = np.zeros((128, NBLK * 2 * CROP), np.float16)
    tr = inputs["theta_re"][bidx]  # [8, 257, 257]
    ti = inputs["theta_im"][bidx]
    for b in range(NBLK):
        ninn = 6 if b < NBLK - 1 else 5
        base = b * 2 * CROP
        for inn in range(ninn):
            row = 6 * b + inn
            for ch in range(CH):
                p = inn * 8 + ch
                th[p, base:base + CROP] = tr[ch, row]
                th[64 + p, base:base + CROP] = tr[ch, row]
                th[p, base + CROP:base + 2 * CROP] = -ti[ch, row]
                th[64 + p, base + CROP:base + 2 * CROP] = ti[ch, row]
    s["thet"] = th
    return s


# ----------------------------------------------------------------------------
# Device program
# ----------------------------------------------------------------------------

def _build_nc():
    nc = bacc.Bacc(None, target_bir_lowering=False, debug=False)

    dp = {}
    for name, shape, dt in (
        ("r16", [N1, N1], F16), ("g1t", [N1, CROP], F16),
        ("g2t", [N1, CROP], F16), ("hrt", [CROP, N1], F16),
        ("hit", [CROP, N1], F16), ("hnit", [CROP, N1], F16),
        ("t1", [8, 384], F16), ("t2", [128, 384], F16),
        ("t2b", [128, 384], F16), ("t3", [128, 384], F16),
        ("t3b", [128, 384], F16),
        ("t3s", [128, 384], F16), ("t3sb", [128, 384], F16), ("t4", [128, 384], F16),
        ("t4b", [128, 384], F16), ("t5", [128, 384], F16),
        ("t5b", [128, 384], F16), ("t6", [128, 36], F16),
        ("t6b", [128, 36], F16), ("thet", [128, NBLK * 2 * CROP], F16),
    ):
        dp[name] = nc.declare_dram_parameter(name, list(shape), dt,
                                             isOutput=False)
    ere = nc.declare_dram_parameter("ere", [N1, N1], F32, isOutput=True)
    eim = nc.declare_dram_parameter("eim", [N1, N1], F32, isOutput=True)

    with TileContext(nc) as tc:
        with (
            tc.tile_pool(name="const", bufs=1) as pc,
            tc.tile_pool(name="xbuf", bufs=1) as px,
            tc.tile_pool(name="work", bufs=1) as pw,
            tc.tile_pool(name="wk2", bufs=3) as pw2,
            tc.tile_pool(name="psum", bufs=8, space="PSUM") as pp,
        ):
            # ---------------- constant loads ----------------
            def load_const(name):
                shape = [int(x) for x in dp[name].shape]
                t = pc.tile(shape, F16, name=name, tag=name)
                nc.sync.dma_start(t[:, :], dp[name][:, :])
                return t

            def load_chunks(name, rows, cols):
                out = []
                r0 = 0
                while r0 < rows:
                    rr = min(128, rows - r0)
                    t = pc.tile([rr, cols], F16, name=f"{name}{r0}",
                                tag=f"{name}{r0}")
                    nc.sync.dma_start(t[:, :], dp[name][r0:r0 + rr, :])
                    out.append(t)
                    r0 += rr
                return out

            r_sb = load_chunks("r16", N1, N1)            # [128,255],[127,255]
            g1_sb = load_chunks("g1t", N1, CROP)
            g2_sb = load_chunks("g2t", N1, CROP)
            hr_sb = load_chunks("hrt", CROP, N1)         # [128],[128],[1]
            hi_sb = load_chunks("hit", CROP, N1)
            hn_sb = load_chunks("hnit", CROP, N1)
            tsb = {k: load_const(k) for k in
                   ("t1", "t2", "t2b", "t3", "t3b", "t4", "t4b",
                    "t5", "t5b", "t6", "t6b", "t3s", "t3sb")}

            # ---------------- X block buffers (write-once) ----------------
            x1 = [px.tile([8, WPAD], F16, name=f"x1_{b}", tag=f"x1_{b}")
                  for b in range(NBLK)]
            xs = {}
            for li in ("2", "3", "4", "5", "6"):
                xs[li] = [px.tile([128, WPAD], F16, name=f"x{li}_{b}",
                                  tag=f"x{li}_{b}") for b in range(NBLK)]

            # predication mask for il7-halo copies: 1 on [112:128], 0 on [96:112]
            mk = pw.tile([128, CROP], U8, name="mk", tag="mk")
            nc.gpsimd.memset(mk[96:128, :], 1.0)
            nc.gpsimd.memset(mk[96:112, :], 0.0)

            for b in range(NBLK):
                nc.vector.memset(x1[b][:, :], 0.0)
            for li, tiles in xs.items():
                for b in range(NBLK):
                    nc.gpsimd.memset(tiles[b][:, 0:259:258], 0.0)
                if li == "4":
                    # il0 zones live at [48:64]; start-48 is illegal so clear
                    # [32:64] (the [32:48] part is later overwritten by theta)
                    nc.gpsimd.memset(tiles[0][32:64, :], 0.0)
                else:
                    nc.gpsimd.memset(tiles[0][96:112, :], 0.0)  # il0 of blk 0
                # block 42: il6/il7 zones are never written (T*b zeroes their
                # weights); full memset keeps reads initialized
                nc.vector.memset(tiles[NBLK - 1][:, :], 0.0)

            # ---------------- front transform ----------------
            # Vt = r^T @ G1^T = (G1 r)^T   [255, 257]
            vt_sb = [pw.tile([128, CROP], F16, name="vt0", tag="vt0"),
                     pw.tile([127, CROP], F16, name="vt1", tag="vt1")]
            for m, (m0, mm) in enumerate(((0, 128), (128, 127))):
                ps = pp.tile([128, CROP], F32, name="ps", tag="ps")
                for k2 in range(2):
                    nc.tensor.matmul(
                        ps[0:mm, :], lhsT=r_sb[k2][:, m0:m0 + mm],
                        rhs=g1_sb[k2][:, :], start=(k2 == 0), stop=(k2 == 1))
                nc.scalar.copy(vt_sb[m][:, :], ps[0:mm, :])

            # rh = Vt^T @ G2^T = G1 r G2^T   [257, 257]
            rh_sb = [pw.tile([128, CROP], F16, name="rh0", tag="rh0"),
                     pw.tile([128, CROP], F16, name="rh1", tag="rh1"),
                     pw.tile([1, CROP], F16, name="rh2", tag="rh2")]
            for m, (m0, mm) in enumerate(((0, 128), (128, 128), (256, 1))):
                ps = pp.tile([128, CROP], F32, name="ps", tag="ps")
                for k2 in range(2):
                    nc.tensor.matmul(
                        ps[0:mm, :], lhsT=vt_sb[k2][:, m0:m0 + mm],
                        rhs=g2_sb[k2][:, :], start=(k2 == 0), stop=(k2 == 1))
                nc.vector.tensor_copy(rh_sb[m][:, :], ps[0:mm, :])

            # scatter rh rows into conv1 input blocks
            for b in range(NBLK):
                lo = max(0, 6 * b - 1)
                hi = min(256, 6 * b + 6)
                r0 = lo
                while r0 <= hi:
                    c = r0 // 128
                    c_end = min(hi, c * 128 + 127)
                    cnt = c_end - r0 + 1
                    il0 = r0 - (6 * b - 1)
                    nc.sync.dma_start(
                        x1[b][il0:il0 + cnt, 1:258],
                        rh_sb[c][r0 - c * 128:r0 - c * 128 + cnt, :])
                    r0 = c_end + 1

            # ---------------- conv layers ----------------
            def conv_matmuls(tkey, xin_b, M, b):
                key = tkey + "b" if (b == NBLK - 1 and tkey != "t1") else tkey
                t = tsb[key]
                ps = pp.tile([128, CROP], F32, name="ps", tag="ps")
                for dj in range(3):
                    nc.tensor.matmul(
                        ps[0:M, :], lhsT=t[:, dj * M:(dj + 1) * M],
                        rhs=xin_b[:, dj:dj + CROP],
                        start=(dj == 0), stop=(dj == 2))
                return ps

            def evict_zones(ps, xout, b):
                hi = 96 if b < NBLK - 1 else 80
                # alternate engines per block to double eviction throughput
                if b % 2:
                    nc.scalar.copy(xout[b][0:hi, 1:258], ps[0:hi, :])
                    zeng = nc.vector
                else:
                    nc.vector.tensor_copy(xout[b][0:hi, 1:258], ps[0:hi, :])
                    zeng = nc.scalar
                # il0 of block b+1 (= this block's inn5 dup)
                if b + 1 < NBLK:
                    zeng.copy(xout[b + 1][96:112, 1:258], ps[96:112, :]) \
                        if zeng is nc.scalar else \
                        zeng.tensor_copy(xout[b + 1][96:112, 1:258],
                                         ps[96:112, :])
                # il7 of block b-1 (= this block's inn0 dup, masked; DVE only)
                if b > 0:
                    nc.vector.copy_predicated(xout[b - 1][96:128, 1:258],
                                              mk[96:128, 0:257],
                                              ps[96:128, :])

            def conv_layer(tkey, xin, xout, kin):
                for b in range(NBLK):
                    xi = xin[b][0:kin, :] if kin < 128 else xin[b][:, :]
                    ps = conv_matmuls(tkey, xi, 128, b)
                    evict_zones(ps, xout, b)

            conv_layer("t1", x1, xs["2"], 8)         # conv1
            conv_layer("t2", xs["2"], xs["3"], 128)  # conv2

            # conv3 + theta: xs3 -> xs4 (x4 layout: re [0:48], im [48:96])
            x4 = xs["4"]
            for b in range(NBLK):
                psA = conv_matmuls("t3", xs["3"][b][:, :], 128, b)
                psB = conv_matmuls("t3s", xs["3"][b][:, :], 128, b)
                tht = pw2.tile([128, 2 * CROP], F16, name="tht", tag="tht",
                               bufs=8)
                nc.sync.dma_start(tht[:, :],
                                  dp["thet"][:, b * 2 * CROP:(b + 1) * 2 * CROP])
                # u = psA*thx, v = psB*thy over lanes [0:112] in one pass;
                # signs are baked into thy so re' and im' are both u+v
                u = pw2.tile([112, CROP], F32, name="m1", tag="m1")
                v = pw2.tile([112, CROP], F32, name="m2", tag="m2")
                nc.vector.tensor_mul(u[:, :], psA[0:112, :],
                                     tht[0:112, 0:CROP])
                nc.vector.tensor_mul(v[:, :], psB[0:112, :],
                                     tht[0:112, CROP:2 * CROP])
                nc.vector.tensor_add(x4[b][0:48, 1:258], u[0:48, :], v[0:48, :])
                nc.vector.tensor_add(x4[b][64:112, 1:258], u[64:112, :],
                                     v[64:112, :])
                # halos via small DMAs (x4 layout is lane-incompatible)
                if b > 0:
                    nc.sync.dma_start(x4[b][48:56, 1:258],
                                      x4[b - 1][40:48, 1:258])
                    nc.gpsimd.dma_start(x4[b][56:64, 1:258],
                                        x4[b - 1][104:112, 1:258])
                    nc.sync.dma_start(x4[b - 1][112:120, 1:258],
                                      x4[b][0:8, 1:258])
                    nc.gpsimd.dma_start(x4[b - 1][120:128, 1:258],
                                        x4[b][64:72, 1:258])

            conv_layer("t4", xs["4"], xs["5"], 128)  # conv4
            conv_layer("t5", xs["5"], xs["6"], 128)  # conv5

            # conv6: xs6 -> Xout planes [257, 257] re/im (3 chunks each)
            xo = {}
            for p in ("re", "im"):
                xo[p] = [pw.tile([128, CROP], F16, name=f"xo{p}0", tag=f"xo{p}0"),
                         pw.tile([128, CROP], F16, name=f"xo{p}1", tag=f"xo{p}1"),
                         pw.tile([1, CROP], F16, name=f"xo{p}2", tag=f"xo{p}2")]
            for b in range(NBLK):
                ps = conv_matmuls("t6", xs["6"][b][:, :], 12, b)
                y6 = pw2.tile([12, CROP], F16, name="y6", tag="y6", bufs=4)
                nc.scalar.copy(y6[:, :], ps[0:12, :])
                nil = 6 if b < NBLK - 1 else 5
                for pi, p in enumerate(("re", "im")):
                    r0 = 6 * b
                    while r0 < 6 * b + nil:
                        c = r0 // 128
                        c_end = min(6 * b + nil - 1, c * 128 + 127)
                        cnt = c_end - r0 + 1
                        nc.sync.dma_start(
                            xo[p][c][r0 - c * 128:r0 - c * 128 + cnt, :],
                            y6[pi * 6 + (r0 - 6 * b):pi * 6 + (r0 - 6 * b) + cnt, :])
                        r0 = c_end + 1

            # ---------------- back transform ----------------
            at = {}
            for p in ("re", "im"):
                at[p] = [pw.tile([128, N1], F16, name=f"at{p}0", tag=f"at{p}0"),
                         pw.tile([128, N1], F16, name=f"at{p}1", tag=f"at{p}1"),
                         pw.tile([1, N1], F16, name=f"at{p}2", tag=f"at{p}2")]
            for m, (m0, mm) in enumerate(((0, 128), (128, 128), (256, 1))):
                for p, terms in (("re", (("re", hr_sb), ("im", hn_sb))),
                                 ("im", (("re", hi_sb), ("im", hr_sb)))):
                    ps = pp.tile([128, N1], F32, name="ps", tag="ps")
                    nmm = 0
                    for (xp, hsb) in terms:
                        for k2 in range(3):
                            nc.tensor.matmul(
                                ps[0:mm, :],
                                lhsT=xo[xp][k2][:, m0:m0 + mm],
                                rhs=hsb[k2][:, :],
                                start=(nmm == 0), stop=(nmm == 5))
                            nmm += 1
                    nc.scalar.copy(at[p][m][:, :], ps[0:mm, :])

            e_sb = {}
            for p in ("re", "im"):
                e_sb[p] = [pw.tile([128, N1], F32, name=f"e{p}0", tag=f"e{p}0"),
                           pw.tile([127, N1], F32, name=f"e{p}1", tag=f"e{p}1")]
            for m, (m0, mm) in enumerate(((0, 128), (128, 127))):
                for p, terms in (("re", (("re", hr_sb), ("im", hn_sb))),
                                 ("im", (("re", hi_sb), ("im", hr_sb)))):
                    ps = pp.tile([128, N1], F32, name="ps", tag="ps")
                    nmm = 0
                    for (ap_, hsb) in terms:
                        for k2 in range(3):
                            nc.tensor.matmul(
                                ps[0:mm, :],
                                lhsT=at[ap_][k2][:, m0:m0 + mm],
                                rhs=hsb[k2][:, :],
                                start=(nmm == 0), stop=(nmm == 5))
                            nmm += 1
                    nc.vector.tensor_copy(e_sb[p][m][:, :], ps[0:mm, :])

            for p, dram in (("re", ere), ("im", eim)):
                nc.sync.dma_start(dram[0:128, :], e_sb[p][0][:, :])
                nc.sync.dma_start(dram[128:255, :], e_sb[p][1][:, :])

    nc.finalize()
    return nc


_NC_CACHE = None


def _get_nc():
    global _NC_CACHE
    if _NC_CACHE is None:
        _NC_CACHE = _build_nc()
    return _NC_CACHE


def kernel(**inputs):
    global LAST_EXEC_TIME_NS
    inputs = {k: np.asarray(v) for k, v in inputs.items()}
    consts = _host_consts()
    in_maps = [_host_prep_sample(b, inputs, consts) for b in range(B)]
    nc = _get_nc()
    trace = bool(os.environ.get("BASS_TRACE"))
    res = run_bass_kernel_spmd(nc, in_maps, list(range(B)), trace=trace)
    LAST_EXEC_TIME_NS = res.exec_time_ns
    out = np.zeros((B, 1, N1, N1), np.complex64)
    for b in range(B):
        out[b, 0] = res.results[b]["ere"] + 1j * res.results[b]["eim"]
    return out


# revision 19
# speedup vs baseline: 1.4329x; 1.1841x over previous
"""FNS spectral network kernel for 8x TRN2 NeuronCores (data parallel over batch).

Math (verified vs reference to ~3e-7 in fp64):
  per sample b:
    rh = (-Gi) @ r @ Gi.T          Gi[j,n] = sin(pi*(j-128)*(n+1)/256)/256, [257,255]
    x  = conv1(rh) -> conv2 -> conv3 -> *theta -> conv4 -> conv5 -> conv6
         (3x3 per-sample complex convs, pad=1; conv4..6 use _wT weights)
    e  = H @ x @ H.T               H[k,j] = exp(-2i*pi*k*(j-127)/513), [255,257]

Device mapping (1 sample/core, all weights/transforms host-preprocessed):
  - convs as block-Toeplitz matmuls: image in 43 row-blocks of 8 rows
    (stride 6, 1-row halo each side). Std layout: mid rows il=1..6 at
    partitions (il-1)*16+cp in [0:96] (cp=reim*8+ch), il0 halo at [96:112],
    il7 at [112:128]. Free = 259 cols (zero pad col each side). Per block:
    3 matmuls (dj taps via free-dim shifts), stationary T [128, 3*M].
  - T has 32 duplicated output columns ([96:112]=inn5 dup, [112:128]=inn0
    dup) so halo propagation is done by lane-aligned compute copies from
    PSUM (plain copy for il0, copy_predicated for il7) - no partition-
    shifting DMAs in the conv pipeline.
  - theta: conv3 emits (reim,inn,ch) order; DVE complex-multiplies write
    straight into conv4's input tiles (x4 uses a (reim,il,ch) layout);
    only zi lane-move + 4 small halo DMAs per block remain.
  - block 42 uses T variants with rows for image rows 257/258 zeroed, so
    out-of-image garbage partitions are never observed.
  - front/back transforms: dense matmuls using the (A@B)^T = matmul(lhsT=A,
    rhs=B) identity so no on-device transposes are needed.
  - fp16 operands (validated ~1e-3 rel err), fp32 PSUM accumulate.
  - Bacc (not raw Bass) so multi-wait instructions are legalized for TRN2.
"""

import os

import numpy as np

import concourse.bacc as bacc
import concourse.mybir as mybir
from concourse.bass_utils import run_bass_kernel_spmd
from concourse.tile import TileContext

F16 = mybir.dt.float16
F32 = mybir.dt.float32
U8 = mybir.dt.uint8

B = 8
N1 = 255
CROP = 257
CH = 8
NBLK = 43          # ceil(257/6)
WPAD = 259         # 257 cols + 1 zero col each side

LAST_EXEC_TIME_NS = None


# ----------------------------------------------------------------------------
# Host-side constant / weight preprocessing
# ----------------------------------------------------------------------------

def _host_consts():
    j = np.arange(CROP)[:, None]
    n = np.arange(N1)[None, :]
    Gi = (np.sin(np.pi * (j - 128) * (n + 1) / 256.0) / 256.0).astype(np.float32)
    k = np.arange(N1)[:, None]
    jj = np.arange(CROP)[None, :]
    H = np.exp(-2j * np.pi * k * (jj - 127.0) / 513.0)
    return {
        "g1t": np.ascontiguousarray((-Gi).T.astype(np.float16)),   # [255,257]
        "g2t": np.ascontiguousarray(Gi.T.astype(np.float16)),      # [255,257]
        "hrt": np.ascontiguousarray(H.real.T.astype(np.float16)),  # [257,255]
        "hit": np.ascontiguousarray(H.imag.T.astype(np.float16)),  # [257,255]
        "hnit": np.ascontiguousarray((-H.imag).T.astype(np.float16)),
    }


def _expand_w(wre, wim):
    """[Co,Ci,3,3] complex -> real packed [2Co, 2Ci, 3, 3], part = reim*C+ch."""
    Co, Ci = wre.shape[0], wre.shape[1]
    W = np.zeros((2 * Co, 2 * Ci, 3, 3), np.float32)
    W[:Co, :Ci] = wre
    W[:Co, Ci:] = -wim
    W[Co:, :Ci] = wim
    W[Co:, Ci:] = wre
    return W


def _wT(wre, wim):
    """torch _wT: swap cout/cin, transpose 3x3 kernel, conjugate."""
    wre2 = np.swapaxes(np.swapaxes(wre, 0, 1), -2, -1)
    wim2 = -np.swapaxes(np.swapaxes(wim, 0, 1), -2, -1)
    return wre2, wim2


def _row_std(p):
    if p < 96:
        return 1 + p // 16, p % 16
    if p < 112:
        return 0, p - 96
    return 7, p - 112


def _row_x4(p):
    """x4 (theta output) layout: re mid [0:48], il0 halos [48:64] (re,im),
    im mid [64:112], il7 halos [112:128] (re,im)."""
    if p < 48:
        return 1 + p // 8, p % 8
    if p < 64:
        return 0, p - 48         # [48:56] re ch, [56:64] im ch
    if p < 112:
        q = p - 64
        return 1 + q // 8, 8 + q % 8
    return 7, p - 112            # [112:120] re ch, [120:128] im ch


def _col_std_dup(m):
    """M=128 col map with dup halo outputs: [0:96] mid, [96:112] inn5 dup,
    [112:128] inn0 dup."""
    if m < 96:
        return m // 16, m % 16
    if m < 112:
        return 5, m - 96
    return 0, m - 112


def _build_T(Wexp, rowmap, colmap, K, M, zero42=False):
    T = np.zeros((K, 3 * M), np.float32)
    Cin2 = Wexp.shape[1]
    for p in range(K):
        il, cp = rowmap(p)
        if cp >= Cin2:
            continue
        if zero42 and il >= 6:
            continue
        for dj in range(3):
            for m in range(M):
                co = colmap(m)
                if co is None:
                    continue
                inn, op = co
                di = il - inn
                if 0 <= di <= 2:
                    T[p, dj * M + m] = Wexp[op, cp, di, dj]
    return T.astype(np.float16)


def _host_prep_sample(bidx, inputs, consts):
    s = {}
    s["r16"] = np.ascontiguousarray(inputs["r"][bidx, 0].astype(np.float16))
    s.update(consts)

    w1 = (inputs["w1_re"][bidx], inputs["w1_im"][bidx])  # [8,1,3,3]
    w2 = (inputs["w2_re"][bidx], inputs["w2_im"][bidx])
    w3 = (inputs["w3_re"][bidx], inputs["w3_im"][bidx])

    W1r = _expand_w(*w1)[:, 0:1]          # [16, 1, 3, 3] (input is real rh)
    W2 = _expand_w(*w2)
    W3 = _expand_w(*w3)
    W4 = _expand_w(*_wT(*w3))
    W5 = _expand_w(*_wT(*w2))
    W6 = _expand_w(*_wT(*w1))             # [2, 16, 3, 3]

    def col_c3a(m):
        if m < 48:
            return m // 8, m % 8          # re
        if 64 <= m < 112:
            q = m - 64
            return q // 8, 8 + q % 8      # im
        return None

    def col_c3b(m):
        if m < 48:
            return m // 8, 8 + m % 8      # im
        if 64 <= m < 112:
            q = m - 64
            return q // 8, q % 8          # re
        return None

    def col_c6(m):
        return m % 6, m // 6

    def row_x1(p):
        return p, 0

    s["t1"] = _build_T(W1r, row_x1, _col_std_dup, 8, 128)
    s["t2"] = _build_T(W2, _row_std, _col_std_dup, 128, 128)
    s["t2b"] = _build_T(W2, _row_std, _col_std_dup, 128, 128, zero42=True)
    s["t3"] = _build_T(W3, _row_std, col_c3a, 128, 128)
    s["t3b"] = _build_T(W3, _row_std, col_c3a, 128, 128, zero42=True)
    s["t3s"] = _build_T(W3, _row_std, col_c3b, 128, 128)
    s["t3sb"] = _build_T(W3, _row_std, col_c3b, 128, 128, zero42=True)
    s["t4"] = _build_T(W4, _row_x4, _col_std_dup, 128, 128)
    s["t4b"] = _build_T(W4, _row_x4, _col_std_dup, 128, 128, zero42=True)
    s["t5"] = _build_T(W5, _row_std, _col_std_dup, 128, 128)
    s["t5b"] = _build_T(W5, _row_std, _col_std_dup, 128, 128, zero42=True)
    s["t6"] = _build_T(W6, _row_std, col_c6, 128, 12)
    s["t6b"] = _build_T(W6, _row_std, col_c6, 128, 12, zero42=True)

    # theta per-block tiles: [48, NBLK * 2*257], block b re at +0, im at +257;
    # row = inn*8+ch matching conv3's (reim,inn,ch) output order
    th = np.zeros((128, NBLK * 2 * CROP), np.float16)
    tr = inputs["theta_re"][bidx]  # [8, 257, 257]
    ti = inputs["theta_im"][bidx]
    for b in range(NBLK):
        ninn = 6 if b < NBLK - 1 else 5
        base = b * 2 * CROP
        for inn in range(ninn):
            row = 6 * b + inn
            for ch in range(CH):
                p = inn * 8 + ch
                th[p, base:base + CROP] = tr[ch, row]
                th[64 + p, base:base + CROP] = tr[ch, row]
                th[p, base + CROP:base + 2 * CROP] = -ti[ch, row]
                th[64 + p, base + CROP:base + 2 * CROP] = ti[ch, row]
    s["thet"] = th
    return s


# ----------------------------------------------------------------------------
# Device program
# ----------------------------------------------------------------------------

def _build_nc():
    nc = bacc.Bacc(None, target_bir_lowering=False, debug=False)

    dp = {}
    for name, shape, dt in (
        ("r16", [N1, N1], F16), ("g1t", [N1, CROP], F16),
        ("g2t", [N1, CROP], F16), ("hrt", [CROP, N1], F16),
        ("hit", [CROP, N1], F16), ("hnit", [CROP, N1], F16),
        ("t1", [8, 384], F16), ("t2", [128, 384], F16),
        ("t2b", [128, 384], F16), ("t3", [128, 384], F16),
        ("t3b", [128, 384], F16),
        ("t3s", [128, 384], F16), ("t3sb", [128, 384], F16), ("t4", [128, 384], F16),
        ("t4b", [128, 384], F16), ("t5", [128, 384], F16),
        ("t5b", [128, 384], F16), ("t6", [128, 36], F16),
        ("t6b", [128, 36], F16), ("thet", [128, NBLK * 2 * CROP], F16),
    ):
        dp[name] = nc.declare_dram_parameter(name, list(shape), dt,
                                             isOutput=False)
    ere = nc.declare_dram_parameter("ere", [N1, N1], F32, isOutput=True)
    eim = nc.declare_dram_parameter("eim", [N1, N1], F32, isOutput=True)

    with TileContext(nc) as tc:
        with (
            tc.tile_pool(name="const", bufs=1) as pc,
            tc.tile_pool(name="xbuf", bufs=1) as px,
            tc.tile_pool(name="work", bufs=1) as pw,
            tc.tile_pool(name="wk2", bufs=3) as pw2,
            tc.tile_pool(name="psum", bufs=8, space="PSUM") as pp,
        ):
            # ---------------- constant loads ----------------
            def load_const(name):
                shape = [int(x) for x in dp[name].shape]
                t = pc.tile(shape, F16, name=name, tag=name)
                nc.sync.dma_start(t[:, :], dp[name][:, :])
                return t

            def load_chunks(name, rows, cols):
                out = []
                r0 = 0
                while r0 < rows:
                    rr = min(128, rows - r0)
                    t = pc.tile([rr, cols], F16, name=f"{name}{r0}",
                                tag=f"{name}{r0}")
                    nc.sync.dma_start(t[:, :], dp[name][r0:r0 + rr, :])
                    out.append(t)
                    r0 += rr
                return out

            r_sb = load_chunks("r16", N1, N1)            # [128,255],[127,255]
            g1_sb = load_chunks("g1t", N1, CROP)
            g2_sb = load_chunks("g2t", N1, CROP)
            hr_sb = load_chunks("hrt", CROP, N1)         # [128],[128],[1]
            hi_sb = load_chunks("hit", CROP, N1)
            hn_sb = load_chunks("hnit", CROP, N1)
            tsb = {k: load_const(k) for k in
                   ("t1", "t2", "t2b", "t3", "t3b", "t4", "t4b",
                    "t5", "t5b", "t6", "t6b", "t3s", "t3sb")}

            # ---------------- X block buffers (write-once) ----------------
            x1 = [px.tile([8, WPAD], F16, name=f"x1_{b}", tag=f"x1_{b}")
                  for b in range(NBLK)]
            xs = {}
            for li in ("2", "3", "4", "5", "6"):
                xs[li] = [px.tile([128, WPAD], F16, name=f"x{li}_{b}",
                                  tag=f"x{li}_{b}") for b in range(NBLK)]

            # predication mask for il7-halo copies: 1 on [112:128], 0 on [96:112]
            mk = pw.tile([128, CROP], U8, name="mk", tag="mk")
            nc.gpsimd.memset(mk[96:128, :], 1.0)
            nc.gpsimd.memset(mk[96:112, :], 0.0)

            for b in range(NBLK):
                nc.vector.memset(x1[b][:, :], 0.0)
            for li, tiles in xs.items():
                for b in range(NBLK):
                    nc.gpsimd.memset(tiles[b][:, 0:259:258], 0.0)
                if li == "4":
                    # il0 zones live at [48:64]; start-48 is illegal so clear
                    # [32:64] (the [32:48] part is later overwritten by theta)
                    nc.gpsimd.memset(tiles[0][32:64, :], 0.0)
                else:
                    nc.gpsimd.memset(tiles[0][96:112, :], 0.0)  # il0 of blk 0
                # block 42: il6/il7 zones are never written (T*b zeroes their
                # weights); full memset keeps reads initialized
                nc.vector.memset(tiles[NBLK - 1][:, :], 0.0)

            # ---------------- front transform ----------------
            # Vt = r^T @ G1^T = (G1 r)^T   [255, 257]
            vt_sb = [pw.tile([128, CROP], F16, name="vt0", tag="vt0"),
                     pw.tile([127, CROP], F16, name="vt1", tag="vt1")]
            for m, (m0, mm) in enumerate(((0, 128), (128, 127))):
                ps = pp.tile([128, CROP], F32, name="ps", tag="ps")
                for k2 in range(2):
                    nc.tensor.matmul(
                        ps[0:mm, :], lhsT=r_sb[k2][:, m0:m0 + mm],
                        rhs=g1_sb[k2][:, :], start=(k2 == 0), stop=(k2 == 1))
                nc.scalar.copy(vt_sb[m][:, :], ps[0:mm, :])

            # rh = Vt^T @ G2^T = G1 r G2^T   [257, 257]
            rh_sb = [pw.tile([128, CROP], F16, name="rh0", tag="rh0"),
                     pw.tile([128, CROP], F16, name="rh1", tag="rh1"),
                     pw.tile([1, CROP], F16, name="rh2", tag="rh2")]
            for m, (m0, mm) in enumerate(((0, 128), (128, 128), (256, 1))):
                ps = pp.tile([128, CROP], F32, name="ps", tag="ps")
                for k2 in range(2):
                    nc.tensor.matmul(
                        ps[0:mm, :], lhsT=vt_sb[k2][:, m0:m0 + mm],
                        rhs=g2_sb[k2][:, :], start=(k2 == 0), stop=(k2 == 1))
                nc.vector.tensor_copy(rh_sb[m][:, :], ps[0:mm, :])

            # scatter rh rows into conv1 input blocks
            for b in range(NBLK):
                lo = max(0, 6 * b - 1)
                hi = min(256, 6 * b + 6)
                r0 = lo
                while r0 <= hi:
                    c = r0 // 128
                    c_end = min(hi, c * 128 + 127)
                    cnt = c_end - r0 + 1
                    il0 = r0 - (6 * b - 1)
                    (nc.sync if b % 2 else nc.gpsimd).dma_start(
                        x1[b][il0:il0 + cnt, 1:258],
                        rh_sb[c][r0 - c * 128:r0 - c * 128 + cnt, :])
                    r0 = c_end + 1

            # ---------------- conv layers ----------------
            def conv_matmuls(tkey, xin_b, M, b):
                key = tkey + "b" if (b == NBLK - 1 and tkey != "t1") else tkey
                t = tsb[key]
                ps = pp.tile([128, CROP], F32, name="ps", tag="ps")
                for dj in range(3):
                    nc.tensor.matmul(
                        ps[0:M, :], lhsT=t[:, dj * M:(dj + 1) * M],
                        rhs=xin_b[:, dj:dj + CROP],
                        start=(dj == 0), stop=(dj == 2))
                return ps

            def evict_zones(ps, xout, b):
                hi = 96 if b < NBLK - 1 else 80
                # mids on ACT (~50% idle), zones on DVE
                nc.scalar.copy(xout[b][0:hi, 1:258], ps[0:hi, :])
                # il0 of block b+1 (= this block's inn5 dup)
                if b + 1 < NBLK:
                    nc.vector.tensor_copy(xout[b + 1][96:112, 1:258],
                                          ps[96:112, :])
                # il7 of block b-1 (= this block's inn0 dup, masked; DVE only)
                if b > 0:
                    nc.vector.copy_predicated(xout[b - 1][96:128, 1:258],
                                              mk[96:128, 0:257],
                                              ps[96:128, :])

            def conv_layer(tkey, xin, xout, kin):
                for b in range(NBLK):
                    xi = xin[b][0:kin, :] if kin < 128 else xin[b][:, :]
                    ps = conv_matmuls(tkey, xi, 128, b)
                    evict_zones(ps, xout, b)

            conv_layer("t1", x1, xs["2"], 8)         # conv1
            conv_layer("t2", xs["2"], xs["3"], 128)  # conv2

            # conv3 + theta: xs3 -> xs4 (x4 layout: re [0:48], im [48:96])
            x4 = xs["4"]
            for b in range(NBLK):
                psA = conv_matmuls("t3", xs["3"][b][:, :], 128, b)
                psB = conv_matmuls("t3s", xs["3"][b][:, :], 128, b)
                tht = pw2.tile([128, 2 * CROP], F16, name="tht", tag="tht",
                               bufs=8)
                nc.sync.dma_start(tht[:, :],
                                  dp["thet"][:, b * 2 * CROP:(b + 1) * 2 * CROP])
                # u = psA*thx, v = psB*thy over lanes [0:112] in one pass;
                # signs are baked into thy so re' and im' are both u+v
                u = pw2.tile([112, CROP], F32, name="m1", tag="m1")
                v = pw2.tile([112, CROP], F32, name="m2", tag="m2")
                nc.vector.tensor_mul(u[:, :], psA[0:112, :],
                                     tht[0:112, 0:CROP])
                nc.vector.tensor_mul(v[:, :], psB[0:112, :],
                                     tht[0:112, CROP:2 * CROP])
                nc.vector.tensor_add(x4[b][0:48, 1:258], u[0:48, :], v[0:48, :])
                nc.vector.tensor_add(x4[b][64:112, 1:258], u[64:112, :],
                                     v[64:112, :])
                # halos via small DMAs (x4 layout is lane-incompatible)
                if b > 0:
                    nc.sync.dma_start(x4[b][48:56, 1:258],
                                      x4[b - 1][40:48, 1:258])
                    nc.gpsimd.dma_start(x4[b][56:64, 1:258],
                                        x4[b - 1][104:112, 1:258])
                    nc.sync.dma_start(x4[b - 1][112:120, 1:258],
                                      x4[b][0:8, 1:258])
                    nc.gpsimd.dma_start(x4[b - 1][120:128, 1:258],
                                        x4[b][64:72, 1:258])

            conv_layer("t4", xs["4"], xs["5"], 128)  # conv4
            conv_layer("t5", xs["5"], xs["6"], 128)  # conv5

            # conv6: xs6 -> Xout planes [257, 257] re/im (3 chunks each)
            xo = {}
            for p in ("re", "im"):
                xo[p] = [pw.tile([128, CROP], F16, name=f"xo{p}0", tag=f"xo{p}0"),
                         pw.tile([128, CROP], F16, name=f"xo{p}1", tag=f"xo{p}1"),
                         pw.tile([1, CROP], F16, name=f"xo{p}2", tag=f"xo{p}2")]
            for b in range(NBLK):
                ps = conv_matmuls("t6", xs["6"][b][:, :], 12, b)
                y6 = pw2.tile([12, CROP], F16, name="y6", tag="y6", bufs=4)
                nc.scalar.copy(y6[:, :], ps[0:12, :])
                nil = 6 if b < NBLK - 1 else 5
                for pi, p in enumerate(("re", "im")):
                    eng = nc.sync if pi == 0 else nc.gpsimd
                    r0 = 6 * b
                    while r0 < 6 * b + nil:
                        c = r0 // 128
                        c_end = min(6 * b + nil - 1, c * 128 + 127)
                        cnt = c_end - r0 + 1
                        eng.dma_start(
                            xo[p][c][r0 - c * 128:r0 - c * 128 + cnt, :],
                            y6[pi * 6 + (r0 - 6 * b):pi * 6 + (r0 - 6 * b) + cnt, :])
                        r0 = c_end + 1

            # ---------------- back transform ----------------
            at = {}
            for p in ("re", "im"):
                at[p] = [pw.tile([128, N1], F16, name=f"at{p}0", tag=f"at{p}0"),
                         pw.tile([128, N1], F16, name=f"at{p}1", tag=f"at{p}1"),
                         pw.tile([1, N1], F16, name=f"at{p}2", tag=f"at{p}2")]
            for m, (m0, mm) in enumerate(((0, 128), (128, 128), (256, 1))):
                for p, terms in (("re", (("re", hr_sb), ("im", hn_sb))),
                                 ("im", (("re", hi_sb), ("im", hr_sb)))):
                    ps = pp.tile([128, N1], F32, name="ps", tag="ps")
                    nmm = 0
                    for (xp, hsb) in terms:
                        for k2 in range(3):
                            nc.tensor.matmul(
                                ps[0:mm, :],
                                lhsT=xo[xp][k2][:, m0:m0 + mm],
                                rhs=hsb[k2][:, :],
                                start=(nmm == 0), stop=(nmm == 5))
                            nmm += 1
                    nc.scalar.copy(at[p][m][:, :], ps[0:mm, :])

            e_sb = {}
            for p in ("re", "im"):
                e_sb[p] = [pw.tile([128, N1], F32, name=f"e{p}0", tag=f"e{p}0"),
                           pw.tile([127, N1], F32, name=f"e{p}1", tag=f"e{p}1")]
            for m, (m0, mm) in enumerate(((0, 128), (128, 127))):
                for p, terms in (("re", (("re", hr_sb), ("im", hn_sb))),
                                 ("im", (("re", hi_sb), ("im", hr_sb)))):
                    ps = pp.tile([128, N1], F32, name="ps", tag="ps")
                    nmm = 0
                    for (ap_, hsb) in terms:
                        for k2 in range(3):
                            nc.tensor.matmul(
                                ps[0:mm, :],
                                lhsT=at[ap_][k2][:, m0:m0 + mm],
                                rhs=hsb[k2][:, :],
                                start=(nmm == 0), stop=(nmm == 5))
                            nmm += 1
                    nc.vector.tensor_copy(e_sb[p][m][:, :], ps[0:mm, :])

            for p, dram in (("re", ere), ("im", eim)):
                nc.sync.dma_start(dram[0:128, :], e_sb[p][0][:, :])
                nc.sync.dma_start(dram[128:255, :], e_sb[p][1][:, :])

    nc.finalize()
    return nc


_NC_CACHE = None


def _get_nc():
    global _NC_CACHE
    if _NC_CACHE is None:
        _NC_CACHE = _build_nc()
    return _NC_CACHE


def kernel(**inputs):
    global LAST_EXEC_TIME_NS
    inputs = {k: np.asarray(v) for k, v in inputs.items()}
    consts = _host_consts()
    in_maps = [_host_prep_sample(b, inputs, consts) for b in range(B)]
    nc = _get_nc()
    trace = bool(os.environ.get("BASS_TRACE"))
    res = run_bass_kernel_spmd(nc, in_maps, list(range(B)), trace=trace)
    LAST_EXEC_TIME_NS = res.exec_time_ns
    out = np.zeros((B, 1, N1, N1), np.complex64)
    for b in range(B):
        out[b, 0] = res.results[b]["ere"] + 1j * res.results[b]["eim"]
    return out
